# revision 60
# baseline (speedup 1.0000x reference)
"""Sharded causal attention kernel for trn2 (per-core program builder), v8.

Sharding: 8 cores = 2 batches x 4 head-groups (4 heads each); each core
computes its heads' full attention; host sums the two head-group partial
out-projections per batch.

Per-core structure (bf16 matmuls, fp32 psum):
  - rmsnorm overlapped with the x DMA stream: squares split DVE/Act,
    Act abs_rsqrt replaces sqrt+max+reciprocal, ssq chunks packed two
    per PSUM bank (partition offsets 0/64)
  - rotary fused into the projections: packed [D,128] rotate-half
    weights (4 heads x 32 rot dims), per-token rms scale folded into
    the cos/sin multipliers, combined into qT/kT via gpsimd SWDGE
    DMA accumulate-adds (keeps DVE off the critical path)
  - attention: block-causal at 128-token granularity (diagonal tiles
    trimmed), causal mask applied post-exp as a 0/1 multiply on Es,
    softmax denominator produced by a 64-wide ones block appended to v
    (lands replicated on psum rows 64:128 - no broadcast matmul),
    g-loop software-pipelined so Act runs exps back-to-back
  - k-projection chunks and deferred out-projections are interleaved
    into the attention steps as PE filler; out tiles stored as bf16
  - PSUM plan: norm pools (left) free early for psim; projections+po
    share a right-side pool; pvh+drain-po share the remaining banks
"""

from contextlib import ExitStack

import numpy as np

import concourse.bass as bass
import concourse.mybir as mybir
import concourse.tile as tile
from concourse import bacc

f32 = mybir.dt.float32
f32r = mybir.dt.float32r
bf16 = mybir.dt.bfloat16
AF = mybir.ActivationFunctionType
OP = mybir.AluOpType

D = 1024
HPC = 4
DH = 64
ROT = 32
P = 128
NEG = -1e30


def build_program(n=2048, mm_dt="bf16", use_kmask=False):
    KT = D // P
    NQB = n // 512
    NTOK = n // P
    NCH = n // 512
    mdt = {"f32": f32, "f32r": f32r, "bf16": bf16}[mm_dt]
    nc = bacc.Bacc("TRN2", target_bir_lowering=False, debug=False)

    def din(name, shape, dt_):
        return nc.dram_tensor(name, shape, dt_, kind="ExternalInput")

    xT_d = din("xT", [D, n], mdt)
    # weights come in t-major packed layout [128, KT*cols] (one DMA each)
    wq_d = din("wq", [P, KT * HPC * DH], mdt)
    wk_d = din("wk", [P, KT * HPC * DH], mdt)
    wv_d = din("wv", [P, KT * HPC * DH], mdt)
    wqr_d = din("wqr", [P, KT * P], mdt)   # 4 heads x 32 rot cols per t
    wkr_d = din("wkr", [P, KT * P], mdt)
    wo_d = din("wo", [P, 2 * D], mdt)
    cos_d = din("cos128", [P, n], f32)   # rot rows cos, pass rows 1.0
    sin_d = din("sinc128", [P, n], f32)  # all four 32-row blocks = sin
    tri_d = din("tri01", [P, P], mdt)    # 1.0 where key<=query else 0.0
    id_d = din("ident", [P, P], f32)
    km_d = din("kmask", [P, NTOK], f32) if use_kmask else None
    out_d = nc.dram_tensor("out", [n, D], mdt, kind="ExternalOutput")

    with tile.TileContext(nc) as tc, ExitStack() as top:
        persist = top.enter_context(tc.tile_pool(name="persist", bufs=1))
        ones_f32 = persist.tile([P, 1], f32, name="ones_f32")
        nc.vector.memset(ones_f32, 1.0)
        ones_col = persist.tile([P, 1], mdt, name="ones_col")
        nc.vector.tensor_copy(ones_col, ones_f32)
        ones_row_f = persist.tile([1, P], f32, name="ones_row_f")
        nc.vector.memset(ones_row_f, 1.0)
        ones_row = persist.tile([1, P], f32r, name="ones_row")
        nc.vector.tensor_copy(ones_row, ones_row_f)
        # preload the act table containing Square/AbsRsqrt/Copy so the norm
        # path doesn't eat a mid-phase table switch (Exp set loads later once)
        dummy_act = persist.tile([1, 1], f32, name="dummy_act")
        nc.scalar.activation(dummy_act, ones_f32[0:1, 0:1],
                             AF.Abs_reciprocal_sqrt)

        qkv = top.enter_context(tc.tile_pool(name="qkv", bufs=1))
        qT = [qkv.tile([P, n], mdt, name=f"qT{m}", tag=f"qT{m}") for m in range(2)]
        kT = [qkv.tile([P, n], mdt, name=f"kT{m}", tag=f"kT{m}") for m in range(2)]
        # per head: [64 v-dims | 64 ones]; the ones block makes the pv matmul
        # emit the softmax denominator replicated on psum rows 64:128
        v_sb = [qkv.tile([P, HPC * 2 * DH], mdt, name=f"v{tk}", tag=f"v{tk}")
                for tk in range(NTOK)]
        for tk in range(NTOK):
            vv = v_sb[tk].rearrange("p (h c) -> p h c", h=HPC)
            for hh in range(HPC):
                nc.gpsimd.memset(vv[:, hh, DH:2 * DH], 1.0)
        normk = top.enter_context(tc.tile_pool(name="normk", bufs=1))
        rs_col = normk.tile([P, NTOK], f32, name="rs_col")
        late = top.enter_context(tc.tile_pool(name="late", bufs=2))
        wop = top.enter_context(tc.tile_pool(name="wop", bufs=1))

        big = top.enter_context(tc.tile_pool(name="big", bufs=1))
        # DMA issue order = consumption order (single serialized DMA).
        # x0 lands in 512-col chunks so the first square runs ~1.4us earlier.
        x_sb = [big.tile([P, n], mdt, name=f"x{t}", tag=f"x{t}") for t in range(KT)]
        def wload(dsrc, w_, nm):
            tl = big.tile([P, KT * w_], mdt, name=nm, tag=nm)
            nc.sync.dma_start(out=tl, in_=dsrc[:])
            return [tl[:, t * w_:(t + 1) * w_] for t in range(KT)]

        for c in range(NCH):
            nc.sync.dma_start(out=x_sb[0][:, c * 512:(c + 1) * 512],
                              in_=xT_d[0:P, c * 512:(c + 1) * 512])
        for t in range(1, KT):
            nc.sync.dma_start(out=x_sb[t], in_=xT_d[t * P:(t + 1) * P, :])
        wq = wload(wq_d, HPC * DH, "wq")
        cos_sb = big.tile([P, n], f32, name="cos_sb")
        sin_sb = big.tile([P, n], f32, name="sin_sb")
        nc.sync.dma_start(out=cos_sb, in_=cos_d[:])
        nc.sync.dma_start(out=sin_sb, in_=sin_d[:])
        ident_sb = persist.tile([P, P], f32, name="ident_sb")
        nc.sync.dma_start(out=ident_sb, in_=id_d[:])
        wqr = wload(wqr_d, P, "wqr")
        wv = wload(wv_d, HPC * DH, "wv")
        wk = wload(wk_d, HPC * DH, "wk")
        wkr = wload(wkr_d, P, "wkr")
        wo_all = wop.tile([P, 2 * D], mdt, name="wo_all")
        nc.sync.dma_start(out=wo_all, in_=wo_d[:])
        wo_sb = [wo_all[:, m * D:(m + 1) * D] for m in range(2)]
        tri_sb = persist.tile([P, P], mdt, name="tri_sb")
        nc.sync.dma_start(out=tri_sb, in_=tri_d[:])
        if use_kmask:
            km_sb = persist.tile([P, NTOK], f32, name="km_sb")
            nc.sync.dma_start(out=km_sb, in_=km_d[:])

        usp = top.enter_context(tc.tile_pool(name="usp", bufs=3))

        def rot_combine(base, nm_, c, pss, psr):
            sl = slice(c * 512, (c + 1) * 512)
            for m in range(2):
                nc.vector.tensor_mul(base[m][:, sl], pss[m], cos_sb[:, sl])
            nc.vector.tensor_mul(psr, psr, sin_sb[:, sl])
            u = usp.tile([P, 512], mdt, name=f"u_{nm_}{c}", tag="u")
            # k chunks run during attention where Act is exp-bound: keep
            # the Act queue clear there and stage via DVE instead
            if nm_ == "k":
                nc.vector.tensor_copy(u, psr)
            else:
                nc.scalar.copy(u, psr)
            for h in range(HPC):
                m, h2 = h // 2, h % 2
                nc.gpsimd.dma_start(
                    out=base[m][64 * h2:64 * h2 + ROT, sl],
                    in_=u[ROT * h:ROT * (h + 1), :],
                    accum_op=OP.add)

        # ---- rmsnorm + q/v projections: pnorm/pbc on the left free early
        # so psim/pvp land there; pp (bufs=3) closes before attention ----
        with tc.tile_pool(name="pnorm", bufs=1, space="PSUM") as pnorm, \
             tc.tile_pool(name="pbc", bufs=1, space="PSUM") as pbc, \
             tc.tile_pool(name="pproj", bufs=4, side="right",
                          space="PSUM") as pp, \
             tc.tile_pool(name="normt", bufs=1) as normt, \
             tc.tile_pool(name="sqp", bufs=2) as sqp:
            # matmul PSUM writes must start at partition 0/32/64: two
            # chunks per bank at partitions {0, 64}
            ssq2 = [pnorm.tile([P, 512], f32, name=f"ssq2_{i}", tag=f"ssq2_{i}")
                    for i in range(2)]
            sloc = [(ssq2[c // 2], 64 * (c % 2)) for c in range(NCH)]
            for t in range(KT):
                for c in range(NCH):
                    sq = sqp.tile([P, 512], mdt, name=f"sq{t}_{c}", tag="sq")
                    xs = x_sb[t][:, c * 512:(c + 1) * 512]
                    if (t * NCH + c) % 2 == 0:
                        nc.vector.tensor_mul(sq, xs, xs)
                    else:
                        nc.scalar.activation(sq, xs, AF.Square)
                    stile, soff = sloc[c]
                    nc.tensor.matmul(stile[soff:soff + 1, :], ones_col, sq,
                                     start=(t == 0), stop=(t == KT - 1))
            s_row = normt.tile([1, n], f32r, name="s_row")
            for c in range(NCH):
                sl = slice(c * 512, (c + 1) * 512)
                stile, soff = sloc[c]
                # s = 1/sqrt(ssq/D); matches 1/max(sqrt(.), eps) for all
                # realistic (nonzero) token rows
                with nc.allow_low_precision(reason="f32r has f32 bits"):
                    nc.scalar.activation(s_row[:, sl], stile[soff:soff + 1, :],
                                         AF.Abs_reciprocal_sqrt, scale=1.0 / D)
                bc = pbc.tile([P, 512], f32, name=f"bc{c}", tag="bc")
                nc.tensor.matmul(bc, ones_row, s_row[:, sl],
                                 start=True, stop=True)
                nc.vector.tensor_mul(cos_sb[:, sl], cos_sb[:, sl], bc)
                nc.vector.tensor_mul(sin_sb[:, sl], sin_sb[:, sl], bc)
                for tb in range(4):
                    tk = c * 4 + tb
                    dg = sqp.tile([P, P], f32, name=f"dg_{tk}", tag="dg")
                    nc.vector.tensor_mul(dg, bc[:, tb * P:(tb + 1) * P], ident_sb)
                    nc.vector.reduce_sum(rs_col[:, tk:tk + 1], dg,
                                         axis=mybir.AxisListType.X)

            # ---- q projection ----
            for c in range(NCH):
                sl = slice(c * 512, (c + 1) * 512)
                pss = []
                for m in range(2):
                    ps = pp.tile([P, 512], f32, name=f"ppq{m}_{c}", tag="pp")
                    for t in range(KT):
                        nc.tensor.matmul(ps, wq[t][:, m * P:(m + 1) * P],
                                         x_sb[t][:, sl],
                                         start=(t == 0), stop=(t == KT - 1))
                    pss.append(ps)
                psr = pp.tile([P, 512], f32, name=f"ppqr_{c}", tag="pp")
                for t in range(KT):
                    nc.tensor.matmul(psr, wqr[t], x_sb[t][:, sl],
                                     start=(t == 0), stop=(t == KT - 1))
                rot_combine(qT, "q", c, pss, psr)
            # ---- v projection (group 0 only needs tiles 0..3; the rest
            # ride along as attention-step fillers from the ppk pool) ----
            for tk in range(4):
                ps = pp.tile([P, HPC * DH], f32, name=f"ppv_{tk}", tag="pp")
                for t in range(KT):
                    nc.tensor.matmul(ps, x_sb[t][:, tk * P:(tk + 1) * P], wv[t],
                                     start=(t == 0), stop=(t == KT - 1))
                vv = v_sb[tk].rearrange("p (h c) -> p h c", h=HPC)
                nc.vector.tensor_scalar_mul(
                    vv[:, :, 0:DH], ps.rearrange("p (h c) -> p h c", h=HPC),
                    rs_col[:, tk:tk + 1])

        # ---- attention + k-projection, finely interleaved ----
        with tc.tile_pool(name="ep", bufs=3) as ep, \
             tc.tile_pool(name="rcpp", bufs=2) as rcpp, \
             tc.tile_pool(name="outsb", bufs=4) as osb, \
             tc.tile_pool(name="psim", bufs=1, space="PSUM") as psim, \
             tc.tile_pool(name="pvp", bufs=2, space="PSUM") as pvp, \
             tc.tile_pool(name="ppk", bufs=2, space="PSUM") as ppk:
            # all AbsRsqrt uses are behind us: preload the Exp table now so
            # the first attention exp doesn't stall on a mid-pipeline load
            nc.scalar.activation(dummy_act, dummy_act, AF.Exp)

            def v_pieces(tks):
                def piece(tk):
                    def go():
                        ps = ppk.tile([P, HPC * DH], f32, name=f"ppv_{tk}",
                                      tag="ppk")
                        for t in range(KT):
                            nc.tensor.matmul(ps, x_sb[t][:, tk * P:(tk + 1) * P],
                                             wv[t],
                                             start=(t == 0), stop=(t == KT - 1))
                        vv = v_sb[tk].rearrange("p (h c) -> p h c", h=HPC)
                        nc.vector.tensor_scalar_mul(
                            vv[:, :, 0:DH],
                            ps.rearrange("p (h c) -> p h c", h=HPC),
                            rs_col[:, tk:tk + 1])
                    return go
                return [piece(tk) for tk in tks]

            def k_pieces(c):
                sl = slice(c * 512, (c + 1) * 512)
                state = {}

                def piece_m(m):
                    def go():
                        ps = ppk.tile([P, 512], f32, name=f"ppk{m}_{c}",
                                      tag="ppk")
                        for t in range(KT):
                            nc.tensor.matmul(ps, wk[t][:, m * P:(m + 1) * P],
                                             x_sb[t][:, sl],
                                             start=(t == 0), stop=(t == KT - 1))
                        state[m] = ps
                    return go

                def piece_rot():
                    psr = ppk.tile([P, 512], f32, name=f"ppkr_{c}", tag="ppk")
                    for t in range(KT):
                        nc.tensor.matmul(psr, wkr[t], x_sb[t][:, sl],
                                         start=(t == 0), stop=(t == KT - 1))
                    rot_combine(kT, "k", c, [state[0], state[1]], psr)

                return [piece_m(0), piece_m(1), piece_rot]

            pvhs = {}
            attns = {}

            def po_pieces(qb):
                attn = attns[qb]

                def piece_tk(tk):
                    def go():
                        tkl = tk - 4 * qb
                        ob = osb.tile([P, D], mdt, name=f"ob_{tk}", tag="ob")
                        for c2 in range(D // 512):
                            # at the drain (last qb) the pvh slots are free:
                            # alternate pools for a 4-deep po rotation
                            pool = pvp if (qb == NQB - 1 and
                                           (2 * tkl + c2) % 2 == 1) else ppk
                            tg = "pv" if pool is pvp else "ppk"
                            po = pool.tile([P, 512], f32, name=f"po_{tk}_{c2}",
                                           tag=tg)
                            for m in range(2):
                                nc.tensor.matmul(
                                    po, attn[m][:, tkl * P:(tkl + 1) * P],
                                    wo_sb[m][:, c2 * 512:(c2 + 1) * 512],
                                    start=(m == 0), stop=(m == 1))
                            obc = ob[:, c2 * 512:(c2 + 1) * 512]
                            # Act only helps at the drain (qb3), where the
                            # exp stream has ended; elsewhere it would delay
                            # exps which pace the attention stretches
                            if qb == NQB - 1 and c2 == 1:
                                nc.scalar.copy(obc, po)
                            else:
                                nc.vector.tensor_copy(obc, po)
                        nc.sync.dma_start(out=out_d[tk * P:(tk + 1) * P, :],
                                          in_=ob)
                    return go

                return [piece_tk(tk) for tk in range(4 * qb, 4 * qb + 4)]

            def emit_sims(qb, pr, g):
                nkt = 4 * qb + 4
                segs, off = [], 0
                for kt_ in (2 * g, 2 * g + 1):
                    if kt_ >= nkt:
                        continue
                    qlo = max(0, kt_ - 4 * qb) * P
                    segs.append((kt_, qlo, off, 512 - qlo))
                    off += 512 - qlo
                sims = [psim.tile([P, off], f32, name=f"s{h2}_{pr}_{qb}_{g}",
                                  tag=f"sim{h2}") for h2 in range(2)]
                for kt_, qlo, o, w in segs:
                    for h2 in range(2):
                        nc.tensor.matmul(
                            sims[h2][:, o:o + w],
                            kT[pr][64 * h2:64 * h2 + 64, kt_ * P:(kt_ + 1) * P],
                            qT[pr][64 * h2:64 * h2 + 64,
                                   qb * 512 + qlo:(qb + 1) * 512],
                            start=True, stop=True, tile_position=(64 * h2, 0))
                if use_kmask:
                    for kt_, qlo, o, w in segs:
                        for h2 in range(2):
                            sl = sims[h2][:, o:o + w]
                            nc.vector.tensor_scalar_add(sl, sl,
                                                        km_sb[:, kt_:kt_ + 1])
                return sims, segs

            def emit_tail(qb, pr):
                pvh = pvhs[(qb, pr)]
                attns.setdefault(qb, [None, None])
                attns[qb][pr] = late.tile([P, 512], mdt, name=f"attn{pr}_{qb}",
                                          tag=f"attn{pr}")
                at = attns[qb][pr]
                for h2 in range(2):
                    rcp = rcpp.tile([DH, 512], f32, name=f"rcp_{pr}_{qb}_{h2}",
                                    tag="rcp")
                    nc.vector.reciprocal(rcp, pvh[h2][DH:2 * DH, :])
                    nc.vector.tensor_tensor(
                        at[64 * h2:64 * h2 + 64, :], pvh[h2][0:DH, :],
                        rcp, OP.mult)

            def run_group(qb, fillers):
                nkt = 4 * qb + 4
                ng = (nkt + 1) // 2
                steps = [(pr, g) for pr in range(2) for g in range(ng)]
                cur = emit_sims(qb, 0, 0)
                for idx, (pr, g) in enumerate(steps):
                    sims, segs = cur
                    w_ = segs[-1][2] + segs[-1][3]
                    Es = [ep.tile([P, w_], mdt, name=f"E{h2}_{pr}_{qb}_{g}",
                                  tag=f"E{h2}") for h2 in range(2)]
                    for h2 in range(2):
                        nc.scalar.activation(Es[h2], sims[h2], AF.Exp)
                    for kt_, qlo, o, w in segs:
                        if kt_ - 4 * qb >= 0:
                            for h2 in range(2):
                                sl = Es[h2][:, o:o + P]
                                nc.vector.tensor_mul(sl, sl, tri_sb)
                    if idx + 1 < len(steps):
                        cur = emit_sims(qb, *steps[idx + 1])
                    if fillers:
                        fillers.pop(0)()
                    if g == 0:
                        pvhs[(qb, pr)] = [
                            pvp.tile([P, 512], f32, name=f"pv_{pr}_{qb}_{h2}",
                                     tag="pv") for h2 in range(2)]
                    pvh = pvhs[(qb, pr)]
                    for kt_, qlo, o, w in segs:
                        for h2 in range(2):
                            hh = 2 * pr + h2
                            nc.tensor.matmul(
                                pvh[h2][:, qlo:512],
                                v_sb[kt_][:, 2 * DH * hh:2 * DH * hh + 2 * DH],
                                Es[h2][:, o:o + w],
                                start=(kt_ == 0), stop=(kt_ == nkt - 1),
                                skip_group_check=True)
                    if g == ng - 1:
                        emit_tail(qb, pr)
                for f in fillers:
                    f()
                fillers.clear()

            for p_ in k_pieces(0):
                p_()
            run_group(0, k_pieces(1) + v_pieces(range(4, 8)))
            run_group(1, k_pieces(2) + v_pieces(range(8, 12)) + po_pieces(0))
            run_group(2, k_pieces(3) + v_pieces(range(12, 16)) + po_pieces(1))
            run_group(3, po_pieces(2))
            for p_ in po_pieces(3):
                p_()

    nc.compile()
    return nc


# ---------------------------------------------------------------- host side

def np_dt(mm_dt):
    import ml_dtypes
    return {"f32": np.float32, "f32r": np.float32, "bf16": ml_dtypes.bfloat16}[mm_dt]


def _tmajor(W):
    """[D, cols] -> [128, KT*cols] t-major packing for single-DMA load."""
    KT = W.shape[0] // P
    return np.concatenate([W[t * P:(t + 1) * P, :] for t in range(KT)], axis=1)


def make_core_inputs(x, mask, pos_emb, g, Wq, Wkv, Wo, core, n, mm_dt="bf16"):
    ndt = np_dt(mm_dt)
    b = core // 4
    h0 = (core % 4) * HPC
    scale = DH ** -0.5
    gW = Wq * g[:, None]
    gKV = Wkv * g[:, None]
    cols = slice(h0 * DH, (h0 + HPC) * DH)
    wq = gW[:, cols] * scale
    Wk_full = gKV[:, :D]
    Wv_full = gKV[:, D:]
    wk = Wk_full[:, cols]
    wv = Wv_full[:, cols]

    def rot_cols(W):
        # [h0:32 | h1:32 | h2:32 | h3:32] rotate-half columns
        out = np.zeros((D, P), dtype=W.dtype)
        for h in range(HPC):
            src = W[:, (h0 + h) * DH:(h0 + h) * DH + DH]
            base = h * ROT
            out[:, base:base + 16] = -src[:, 16:32]
            out[:, base + 16:base + 32] = src[:, 0:16]
        return out

    wqr = rot_cols(gW) * scale
    wkr = rot_cols(Wk_full)
    wo = np.concatenate([Wo[cols, :][m * P:(m + 1) * P, :] for m in range(2)],
                        axis=1)

    cosf = np.cos(pos_emb.T).astype(np.float32)
    sinf = np.sin(pos_emb.T).astype(np.float32)
    cos128 = np.ones((P, n), np.float32)
    cos128[0:ROT] = cosf
    cos128[DH:DH + ROT] = cosf
    sinc128 = np.empty((P, n), np.float32)
    for h in range(HPC):
        sinc128[h * ROT:(h + 1) * ROT] = sinf
    tri01 = (np.arange(P)[:, None] <= np.arange(P)[None, :]).astype(np.float32)

    ins = {
        "xT": np.ascontiguousarray(x[b].T).astype(ndt),
        "wq": _tmajor(wq).astype(ndt), "wk": _tmajor(wk).astype(ndt),
        "wv": _tmajor(wv).astype(ndt), "wqr": _tmajor(wqr).astype(ndt),
        "wkr": _tmajor(wkr).astype(ndt), "wo": wo.astype(ndt),
        "cos128": cos128, "sinc128": sinc128, "tri01": tri01.astype(ndt),
        "ident": np.eye(P, dtype=np.float32),
    }
    if not mask.all():
        km = np.where(mask[b], 0.0, NEG).astype(np.float32)
        ins["kmask"] = np.ascontiguousarray(km.reshape(n // P, P).T)
    return ins


# ---------------------------------------------------------------- runner

import os
import jax


def _run_per_device(nc, in_maps, core_ids):
    """Run the same Bass program independently on each visible device."""
    from concourse.bass2jax import (_bass_exec_p, install_neuronx_cc_hook,
                                    partition_id_tensor)
    install_neuronx_cc_hook()
    partition_name = nc.partition_id_tensor.name if nc.partition_id_tensor else None
    in_names, out_names, out_avals, zero_outs = [], [], [], []
    for alloc in nc.m.functions[0].allocations:
        if not isinstance(alloc, mybir.MemoryLocationSet):
            continue
        name = alloc.memorylocations[0].name
        if alloc.kind == "ExternalInput":
            if name != partition_name:
                in_names.append(name)
        elif alloc.kind == "ExternalOutput":
            out_names.append(name)
            shape = tuple(alloc.tensor_shape)
            dtype = mybir.dt.np(alloc.dtype)
            out_avals.append(jax.core.ShapedArray(shape, dtype))
            zero_outs.append(np.zeros(shape, dtype))
    n_params = len(in_names)
    all_in_names = list(in_names) + list(out_names)
    if partition_name is not None:
        all_in_names.append(partition_name)
    donate = tuple(range(n_params, n_params + len(out_names)))

    def _body(*args):
        operands = list(args)
        if partition_name is not None:
            operands.append(partition_id_tensor())
        outs = _bass_exec_p.bind(
            *operands, out_avals=tuple(out_avals), in_names=tuple(all_in_names),
            out_names=tuple(out_names), lowering_input_output_aliases=(),
            sim_require_finite=True, sim_require_nnan=True, nc=nc)
        return tuple(outs)

    fn = jax.jit(_body, donate_argnums=donate, keep_unused=True)
    futures = []
    for c, in_map in zip(core_ids, in_maps):
        dev = jax.devices()[c]
        args = [jax.device_put(np.asarray(in_map[nm]), dev) for nm in in_names]
        zz = [jax.device_put(z, dev) for z in zero_outs]
        futures.append(fn(*args, *zz))
    return [{nm: np.asarray(a) for nm, a in zip(out_names, f)} for f in futures]


_PROGRAM_CACHE = {}

MM_DT = "bf16"


def kernel(**inputs):
    os.environ.setdefault("NEURON_COMPILE_CACHE_URL", "/tmp/neuron_cache_kernel")
    x = np.asarray(inputs["x"], dtype=np.float32)
    mask = np.asarray(inputs["mask"]).astype(bool)
    pos_emb = np.asarray(inputs["pos_emb"], dtype=np.float32)
    g = np.asarray(inputs["g"], dtype=np.float32)
    Wq = np.asarray(inputs["Wq"], dtype=np.float32)
    Wkv = np.asarray(inputs["Wkv"], dtype=np.float32)
    Wo = np.asarray(inputs["Wo"], dtype=np.float32)
    bo = np.asarray(inputs["bo"], dtype=np.float32)
    b, n, _ = x.shape
    assert (b, n) == (2, 2048), (b, n)
    mm_dt = MM_DT
    use_km = not bool(mask.all())
    key = (n, mm_dt, use_km)
    if key not in _PROGRAM_CACHE:
        _PROGRAM_CACHE[key] = build_program(n=n, mm_dt=mm_dt, use_kmask=use_km)
    nc = _PROGRAM_CACHE[key]
    core_ids = list(range(8))
    in_maps = [make_core_inputs(x, mask, pos_emb, g, Wq, Wkv, Wo, c, n, mm_dt)
               for c in core_ids]
    results = _run_per_device(nc, in_maps, core_ids)
    out = np.zeros((b, n, D), np.float32)
    for c in core_ids:
        out[c // 4] += results[c]["out"].astype(np.float32)
    out += bo[None, None, :]
    return out


# revision 61
# speedup vs baseline: 1.0010x; 1.0010x over previous
"""Sharded causal attention kernel for trn2 (per-core program builder), v8.

Sharding: 8 cores = 2 batches x 4 head-groups (4 heads each); each core
computes its heads' full attention; host sums the two head-group partial
out-projections per batch.

Per-core structure (bf16 matmuls, fp32 psum):
  - rmsnorm overlapped with the x DMA stream: squares split DVE/Act,
    Act abs_rsqrt replaces sqrt+max+reciprocal, ssq chunks packed two
    per PSUM bank (partition offsets 0/64)
  - rotary fused into the projections: packed [D,128] rotate-half
    weights (4 heads x 32 rot dims), per-token rms scale folded into
    the cos/sin multipliers, combined into qT/kT via gpsimd SWDGE
    DMA accumulate-adds (keeps DVE off the critical path)
  - attention: block-causal at 128-token granularity (diagonal tiles
    trimmed), causal mask applied post-exp as a 0/1 multiply on Es,
    softmax denominator produced by a 64-wide ones block appended to v
    (lands replicated on psum rows 64:128 - no broadcast matmul),
    g-loop software-pipelined so Act runs exps back-to-back
  - k-projection chunks and deferred out-projections are interleaved
    into the attention steps as PE filler; out tiles stored as bf16
  - PSUM plan: norm pools (left) free early for psim; projections+po
    share a right-side pool; pvh+drain-po share the remaining banks
"""

from contextlib import ExitStack

import numpy as np

import concourse.bass as bass
import concourse.mybir as mybir
import concourse.tile as tile
from concourse import bacc

f32 = mybir.dt.float32
f32r = mybir.dt.float32r
bf16 = mybir.dt.bfloat16
AF = mybir.ActivationFunctionType
OP = mybir.AluOpType

D = 1024
HPC = 4
DH = 64
ROT = 32
P = 128
NEG = -1e30


def build_program(n=2048, mm_dt="bf16", use_kmask=False):
    KT = D // P
    NQB = n // 512
    NTOK = n // P
    NCH = n // 512
    mdt = {"f32": f32, "f32r": f32r, "bf16": bf16}[mm_dt]
    nc = bacc.Bacc("TRN2", target_bir_lowering=False, debug=False)

    def din(name, shape, dt_):
        return nc.dram_tensor(name, shape, dt_, kind="ExternalInput")

    xT_d = din("xT", [D, n], mdt)
    # weights come in t-major packed layout [128, KT*cols] (one DMA each)
    wq_d = din("wq", [P, KT * HPC * DH], mdt)
    wk_d = din("wk", [P, KT * HPC * DH], mdt)
    wv_d = din("wv", [P, KT * HPC * DH], mdt)
    wqr_d = din("wqr", [P, KT * P], mdt)   # 4 heads x 32 rot cols per t
    wkr_d = din("wkr", [P, KT * P], mdt)
    wo_d = din("wo", [P, 2 * D], mdt)
    cos_d = din("cos128", [P, n], f32)   # rot rows cos, pass rows 1.0
    sin_d = din("sinc128", [P, n], f32)  # all four 32-row blocks = sin
    tri_d = din("tri01", [P, P], mdt)    # 1.0 where key<=query else 0.0
    id_d = din("ident", [P, P], f32)
    km_d = din("kmask", [P, NTOK], f32) if use_kmask else None
    out_d = nc.dram_tensor("out", [n, D], mdt, kind="ExternalOutput")

    with tile.TileContext(nc) as tc, ExitStack() as top:
        persist = top.enter_context(tc.tile_pool(name="persist", bufs=1))
        ones_f32 = persist.tile([P, 1], f32, name="ones_f32")
        nc.vector.memset(ones_f32, 1.0)
        ones_col = persist.tile([P, 1], mdt, name="ones_col")
        nc.vector.tensor_copy(ones_col, ones_f32)
        ones_row_f = persist.tile([1, P], f32, name="ones_row_f")
        nc.vector.memset(ones_row_f, 1.0)
        ones_row = persist.tile([1, P], f32r, name="ones_row")
        nc.vector.tensor_copy(ones_row, ones_row_f)
        # preload the act table containing Square/AbsRsqrt/Copy so the norm
        # path doesn't eat a mid-phase table switch (Exp set loads later once)
        dummy_act = persist.tile([1, 1], f32, name="dummy_act")
        nc.scalar.activation(dummy_act, ones_f32[0:1, 0:1],
                             AF.Abs_reciprocal_sqrt)

        qkv = top.enter_context(tc.tile_pool(name="qkv", bufs=1))
        qT = [qkv.tile([P, n], mdt, name=f"qT{m}", tag=f"qT{m}") for m in range(2)]
        kT = [qkv.tile([P, n], mdt, name=f"kT{m}", tag=f"kT{m}") for m in range(2)]
        # per head: [64 v-dims | 64 ones]; the ones block makes the pv matmul
        # emit the softmax denominator replicated on psum rows 64:128
        v_sb = [qkv.tile([P, HPC * 2 * DH], mdt, name=f"v{tk}", tag=f"v{tk}")
                for tk in range(NTOK)]
        for tk in range(NTOK):
            vv = v_sb[tk].rearrange("p (h c) -> p h c", h=HPC)
            for hh in range(HPC):
                nc.gpsimd.memset(vv[:, hh, DH:2 * DH], 1.0)
        normk = top.enter_context(tc.tile_pool(name="normk", bufs=1))
        rs_col = normk.tile([P, NTOK], f32, name="rs_col")
        late = top.enter_context(tc.tile_pool(name="late", bufs=2))
        wop = top.enter_context(tc.tile_pool(name="wop", bufs=1))

        big = top.enter_context(tc.tile_pool(name="big", bufs=1))
        # DMA issue order = consumption order (single serialized DMA).
        # x0 lands in 512-col chunks so the first square runs ~1.4us earlier.
        x_sb = [big.tile([P, n], mdt, name=f"x{t}", tag=f"x{t}") for t in range(KT)]
        def wload(dsrc, w_, nm):
            tl = big.tile([P, KT * w_], mdt, name=nm, tag=nm)
            nc.sync.dma_start(out=tl, in_=dsrc[:])
            return [tl[:, t * w_:(t + 1) * w_] for t in range(KT)]

        for c in range(NCH):
            nc.sync.dma_start(out=x_sb[0][:, c * 512:(c + 1) * 512],
                              in_=xT_d[0:P, c * 512:(c + 1) * 512])
        for t in range(1, KT):
            nc.sync.dma_start(out=x_sb[t], in_=xT_d[t * P:(t + 1) * P, :])
        wq = wload(wq_d, HPC * DH, "wq")
        cos_sb = big.tile([P, n], f32, name="cos_sb")
        sin_sb = big.tile([P, n], f32, name="sin_sb")
        nc.sync.dma_start(out=cos_sb, in_=cos_d[:])
        nc.sync.dma_start(out=sin_sb, in_=sin_d[:])
        ident_sb = persist.tile([P, P], f32, name="ident_sb")
        nc.sync.dma_start(out=ident_sb, in_=id_d[:])
        wqr = wload(wqr_d, P, "wqr")
        wv = wload(wv_d, HPC * DH, "wv")
        wk = wload(wk_d, HPC * DH, "wk")
        wkr = wload(wkr_d, P, "wkr")
        wo_all = wop.tile([P, 2 * D], mdt, name="wo_all")
        nc.sync.dma_start(out=wo_all, in_=wo_d[:])
        wo_sb = [wo_all[:, m * D:(m + 1) * D] for m in range(2)]
        tri_sb = persist.tile([P, P], mdt, name="tri_sb")
        nc.sync.dma_start(out=tri_sb, in_=tri_d[:])
        if use_kmask:
            km_sb = persist.tile([P, NTOK], f32, name="km_sb")
            nc.sync.dma_start(out=km_sb, in_=km_d[:])

        usp = top.enter_context(tc.tile_pool(name="usp", bufs=3))

        def rot_combine(base, nm_, c, pss, psr):
            sl = slice(c * 512, (c + 1) * 512)
            for m in range(2):
                nc.vector.tensor_mul(base[m][:, sl], pss[m], cos_sb[:, sl])
            nc.vector.tensor_mul(psr, psr, sin_sb[:, sl])
            u = usp.tile([P, 512], mdt, name=f"u_{nm_}{c}", tag="u")
            # k chunks run during attention where Act is exp-bound: keep
            # the Act queue clear there and stage via DVE instead
            if nm_ == "k":
                nc.vector.tensor_copy(u, psr)
            else:
                nc.scalar.copy(u, psr)
            for h in range(HPC):
                m, h2 = h // 2, h % 2
                nc.gpsimd.dma_start(
                    out=base[m][64 * h2:64 * h2 + ROT, sl],
                    in_=u[ROT * h:ROT * (h + 1), :],
                    accum_op=OP.add)

        # ---- rmsnorm + q/v projections: pnorm/pbc on the left free early
        # so psim/pvp land there; pp (bufs=3) closes before attention ----
        with tc.tile_pool(name="pnorm", bufs=1, space="PSUM") as pnorm, \
             tc.tile_pool(name="pbc", bufs=1, space="PSUM") as pbc, \
             tc.tile_pool(name="pproj", bufs=4, side="right",
                          space="PSUM") as pp, \
             tc.tile_pool(name="normt", bufs=1) as normt, \
             tc.tile_pool(name="sqp", bufs=2) as sqp:
            # matmul PSUM writes must start at partition 0/32/64: two
            # chunks per bank at partitions {0, 64}
            ssq2 = [pnorm.tile([P, 512], f32, name=f"ssq2_{i}", tag=f"ssq2_{i}")
                    for i in range(2)]
            sloc = [(ssq2[c // 2], 64 * (c % 2)) for c in range(NCH)]
            for t in range(KT):
                for c in range(NCH):
                    sq = sqp.tile([P, 512], mdt, name=f"sq{t}_{c}", tag="sq")
                    xs = x_sb[t][:, c * 512:(c + 1) * 512]
                    if (t * NCH + c) % 2 == 0:
                        nc.vector.tensor_mul(sq, xs, xs)
                    else:
                        nc.scalar.activation(sq, xs, AF.Square)
                    stile, soff = sloc[c]
                    nc.tensor.matmul(stile[soff:soff + 1, :], ones_col, sq,
                                     start=(t == 0), stop=(t == KT - 1))
            s_row = normt.tile([1, n], f32r, name="s_row")
            for c in range(NCH):
                sl = slice(c * 512, (c + 1) * 512)
                stile, soff = sloc[c]
                # s = 1/sqrt(ssq/D); matches 1/max(sqrt(.), eps) for all
                # realistic (nonzero) token rows
                with nc.allow_low_precision(reason="f32r has f32 bits"):
                    nc.scalar.activation(s_row[:, sl], stile[soff:soff + 1, :],
                                         AF.Abs_reciprocal_sqrt, scale=1.0 / D)
                bc = pbc.tile([P, 512], f32, name=f"bc{c}", tag="bc")
                nc.tensor.matmul(bc, ones_row, s_row[:, sl],
                                 start=True, stop=True)
                nc.vector.tensor_mul(cos_sb[:, sl], cos_sb[:, sl], bc)
                nc.vector.tensor_mul(sin_sb[:, sl], sin_sb[:, sl], bc)
                for tb in range(4):
                    tk = c * 4 + tb
                    dg = sqp.tile([P, P], f32, name=f"dg_{tk}", tag="dg")
                    nc.vector.tensor_mul(dg, bc[:, tb * P:(tb + 1) * P], ident_sb)
                    nc.vector.reduce_sum(rs_col[:, tk:tk + 1], dg,
                                         axis=mybir.AxisListType.X)

            # ---- q projection ----
            for c in range(NCH):
                sl = slice(c * 512, (c + 1) * 512)
                pss = []
                for m in range(2):
                    ps = pp.tile([P, 512], f32, name=f"ppq{m}_{c}", tag="pp")
                    for t in range(KT):
                        nc.tensor.matmul(ps, wq[t][:, m * P:(m + 1) * P],
                                         x_sb[t][:, sl],
                                         start=(t == 0), stop=(t == KT - 1))
                    pss.append(ps)
                psr = pp.tile([P, 512], f32, name=f"ppqr_{c}", tag="pp")
                for t in range(KT):
                    nc.tensor.matmul(psr, wqr[t], x_sb[t][:, sl],
                                     start=(t == 0), stop=(t == KT - 1))
                rot_combine(qT, "q", c, pss, psr)
            # ---- v projection ----
            for tk in range(NTOK):
                ps = pp.tile([P, HPC * DH], f32, name=f"ppv_{tk}", tag="pp")
                for t in range(KT):
                    nc.tensor.matmul(ps, x_sb[t][:, tk * P:(tk + 1) * P], wv[t],
                                     start=(t == 0), stop=(t == KT - 1))
                vv = v_sb[tk].rearrange("p (h c) -> p h c", h=HPC)
                nc.vector.tensor_scalar_mul(
                    vv[:, :, 0:DH], ps.rearrange("p (h c) -> p h c", h=HPC),
                    rs_col[:, tk:tk + 1])

        # ---- attention + k-projection, finely interleaved ----
        with tc.tile_pool(name="ep", bufs=3) as ep, \
             tc.tile_pool(name="rcpp", bufs=2) as rcpp, \
             tc.tile_pool(name="outsb", bufs=4) as osb, \
             tc.tile_pool(name="psim", bufs=1, space="PSUM") as psim, \
             tc.tile_pool(name="pvp", bufs=2, space="PSUM") as pvp, \
             tc.tile_pool(name="ppk", bufs=2, space="PSUM") as ppk:
            # all AbsRsqrt uses are behind us: preload the Exp table now so
            # the first attention exp doesn't stall on a mid-pipeline load
            nc.scalar.activation(dummy_act, dummy_act, AF.Exp)

            def k_pieces(c):
                sl = slice(c * 512, (c + 1) * 512)
                state = {}

                def piece_m(m):
                    def go():
                        ps = ppk.tile([P, 512], f32, name=f"ppk{m}_{c}",
                                      tag="ppk")
                        for t in range(KT):
                            nc.tensor.matmul(ps, wk[t][:, m * P:(m + 1) * P],
                                             x_sb[t][:, sl],
                                             start=(t == 0), stop=(t == KT - 1))
                        state[m] = ps
                    return go

                def piece_rot():
                    psr = ppk.tile([P, 512], f32, name=f"ppkr_{c}", tag="ppk")
                    for t in range(KT):
                        nc.tensor.matmul(psr, wkr[t], x_sb[t][:, sl],
                                         start=(t == 0), stop=(t == KT - 1))
                    rot_combine(kT, "k", c, [state[0], state[1]], psr)

                return [piece_m(0), piece_m(1), piece_rot]

            pvhs = {}
            attns = {}

            def po_pieces(qb):
                attn = attns[qb]

                def piece_tk(tk):
                    def go():
                        tkl = tk - 4 * qb
                        ob = osb.tile([P, D], mdt, name=f"ob_{tk}", tag="ob")
                        for c2 in range(D // 512):
                            # at the drain (last qb) the pvh slots are free:
                            # alternate pools for a 4-deep po rotation
                            pool = pvp if (qb == NQB - 1 and
                                           (2 * tkl + c2) % 2 == 1) else ppk
                            tg = "pv" if pool is pvp else "ppk"
                            po = pool.tile([P, 512], f32, name=f"po_{tk}_{c2}",
                                           tag=tg)
                            for m in range(2):
                                nc.tensor.matmul(
                                    po, attn[m][:, tkl * P:(tkl + 1) * P],
                                    wo_sb[m][:, c2 * 512:(c2 + 1) * 512],
                                    start=(m == 0), stop=(m == 1))
                            obc = ob[:, c2 * 512:(c2 + 1) * 512]
                            # Act only helps at the drain (qb3), where the
                            # exp stream has ended; elsewhere it would delay
                            # exps which pace the attention stretches
                            if qb == NQB - 1 and c2 == 1:
                                nc.scalar.copy(obc, po)
                            else:
                                nc.vector.tensor_copy(obc, po)
                        nc.sync.dma_start(out=out_d[tk * P:(tk + 1) * P, :],
                                          in_=ob)
                    return go

                return [piece_tk(tk) for tk in range(4 * qb, 4 * qb + 4)]

            def emit_sims(qb, pr, g):
                nkt = 4 * qb + 4
                segs, off = [], 0
                for kt_ in (2 * g, 2 * g + 1):
                    if kt_ >= nkt:
                        continue
                    qlo = max(0, kt_ - 4 * qb) * P
                    segs.append((kt_, qlo, off, 512 - qlo))
                    off += 512 - qlo
                sims = [psim.tile([P, off], f32, name=f"s{h2}_{pr}_{qb}_{g}",
                                  tag=f"sim{h2}") for h2 in range(2)]
                for kt_, qlo, o, w in segs:
                    for h2 in range(2):
                        nc.tensor.matmul(
                            sims[h2][:, o:o + w],
                            kT[pr][64 * h2:64 * h2 + 64, kt_ * P:(kt_ + 1) * P],
                            qT[pr][64 * h2:64 * h2 + 64,
                                   qb * 512 + qlo:(qb + 1) * 512],
                            start=True, stop=True, tile_position=(64 * h2, 0))
                if use_kmask:
                    for kt_, qlo, o, w in segs:
                        for h2 in range(2):
                            sl = sims[h2][:, o:o + w]
                            nc.vector.tensor_scalar_add(sl, sl,
                                                        km_sb[:, kt_:kt_ + 1])
                return sims, segs

            def emit_tail(qb, pr):
                pvh = pvhs[(qb, pr)]
                attns.setdefault(qb, [None, None])
                attns[qb][pr] = late.tile([P, 512], mdt, name=f"attn{pr}_{qb}",
                                          tag=f"attn{pr}")
                at = attns[qb][pr]
                for h2 in range(2):
                    rcp = rcpp.tile([DH, 512], f32, name=f"rcp_{pr}_{qb}_{h2}",
                                    tag="rcp")
                    nc.vector.reciprocal(rcp, pvh[h2][DH:2 * DH, :])
                    nc.vector.tensor_tensor(
                        at[64 * h2:64 * h2 + 64, :], pvh[h2][0:DH, :],
                        rcp, OP.mult)

            def run_group(qb, fillers):
                nkt = 4 * qb + 4
                ng = (nkt + 1) // 2
                steps = [(pr, g) for pr in range(2) for g in range(ng)]
                cur = emit_sims(qb, 0, 0)
                for idx, (pr, g) in enumerate(steps):
                    sims, segs = cur
                    w_ = segs[-1][2] + segs[-1][3]
                    Es = [ep.tile([P, w_], mdt, name=f"E{h2}_{pr}_{qb}_{g}",
                                  tag=f"E{h2}") for h2 in range(2)]
                    for h2 in range(2):
                        nc.scalar.activation(Es[h2], sims[h2], AF.Exp)
                    for kt_, qlo, o, w in segs:
                        if kt_ - 4 * qb >= 0:
                            for h2 in range(2):
                                sl = Es[h2][:, o:o + P]
                                nc.vector.tensor_mul(sl, sl, tri_sb)
                    if idx + 1 < len(steps):
                        cur = emit_sims(qb, *steps[idx + 1])
                    if fillers:
                        fillers.pop(0)()
                    if g == 0:
                        pvhs[(qb, pr)] = [
                            pvp.tile([P, 512], f32, name=f"pv_{pr}_{qb}_{h2}",
                                     tag="pv") for h2 in range(2)]
                    pvh = pvhs[(qb, pr)]
                    for kt_, qlo, o, w in segs:
                        for h2 in range(2):
                            hh = 2 * pr + h2
                            nc.tensor.matmul(
                                pvh[h2][:, qlo:512],
                                v_sb[kt_][:, 2 * DH * hh:2 * DH * hh + 2 * DH],
                                Es[h2][:, o:o + w],
                                start=(kt_ == 0), stop=(kt_ == nkt - 1),
                                skip_group_check=True)
                    if g == ng - 1:
                        emit_tail(qb, pr)
                for f in fillers:
                    f()
                fillers.clear()

            for p_ in k_pieces(0):
                p_()
            run_group(0, k_pieces(1))
            run_group(1, k_pieces(2) + po_pieces(0))
            run_group(2, k_pieces(3) + po_pieces(1))
            run_group(3, po_pieces(2))
            for p_ in po_pieces(3):
                p_()

    nc.compile()
    return nc


# ---------------------------------------------------------------- host side

def np_dt(mm_dt):
    import ml_dtypes
    return {"f32": np.float32, "f32r": np.float32, "bf16": ml_dtypes.bfloat16}[mm_dt]


def _tmajor(W):
    """[D, cols] -> [128, KT*cols] t-major packing for single-DMA load."""
    KT = W.shape[0] // P
    return np.concatenate([W[t * P:(t + 1) * P, :] for t in range(KT)], axis=1)


def make_core_inputs(x, mask, pos_emb, g, Wq, Wkv, Wo, core, n, mm_dt="bf16"):
    ndt = np_dt(mm_dt)
    b = core // 4
    h0 = (core % 4) * HPC
    scale = DH ** -0.5
    gW = Wq * g[:, None]
    gKV = Wkv * g[:, None]
    cols = slice(h0 * DH, (h0 + HPC) * DH)
    wq = gW[:, cols] * scale
    Wk_full = gKV[:, :D]
    Wv_full = gKV[:, D:]
    wk = Wk_full[:, cols]
    wv = Wv_full[:, cols]

    def rot_cols(W):
        # [h0:32 | h1:32 | h2:32 | h3:32] rotate-half columns
        out = np.zeros((D, P), dtype=W.dtype)
        for h in range(HPC):
            src = W[:, (h0 + h) * DH:(h0 + h) * DH + DH]
            base = h * ROT
            out[:, base:base + 16] = -src[:, 16:32]
            out[:, base + 16:base + 32] = src[:, 0:16]
        return out

    wqr = rot_cols(gW) * scale
    wkr = rot_cols(Wk_full)
    wo = np.concatenate([Wo[cols, :][m * P:(m + 1) * P, :] for m in range(2)],
                        axis=1)

    cosf = np.cos(pos_emb.T).astype(np.float32)
    sinf = np.sin(pos_emb.T).astype(np.float32)
    cos128 = np.ones((P, n), np.float32)
    cos128[0:ROT] = cosf
    cos128[DH:DH + ROT] = cosf
    sinc128 = np.empty((P, n), np.float32)
    for h in range(HPC):
        sinc128[h * ROT:(h + 1) * ROT] = sinf
    tri01 = (np.arange(P)[:, None] <= np.arange(P)[None, :]).astype(np.float32)

    ins = {
        "xT": np.ascontiguousarray(x[b].T).astype(ndt),
        "wq": _tmajor(wq).astype(ndt), "wk": _tmajor(wk).astype(ndt),
        "wv": _tmajor(wv).astype(ndt), "wqr": _tmajor(wqr).astype(ndt),
        "wkr": _tmajor(wkr).astype(ndt), "wo": wo.astype(ndt),
        "cos128": cos128, "sinc128": sinc128, "tri01": tri01.astype(ndt),
        "ident": np.eye(P, dtype=np.float32),
    }
    if not mask.all():
        km = np.where(mask[b], 0.0, NEG).astype(np.float32)
        ins["kmask"] = np.ascontiguousarray(km.reshape(n // P, P).T)
    return ins


# ---------------------------------------------------------------- runner

import os
import jax


def _run_per_device(nc, in_maps, core_ids):
    """Run the same Bass program independently on each visible device."""
    from concourse.bass2jax import (_bass_exec_p, install_neuronx_cc_hook,
                                    partition_id_tensor)
    install_neuronx_cc_hook()
    partition_name = nc.partition_id_tensor.name if nc.partition_id_tensor else None
    in_names, out_names, out_avals, zero_outs = [], [], [], []
    for alloc in nc.m.functions[0].allocations:
        if not isinstance(alloc, mybir.MemoryLocationSet):
            continue
        name = alloc.memorylocations[0].name
        if alloc.kind == "ExternalInput":
            if name != partition_name:
                in_names.append(name)
        elif alloc.kind == "ExternalOutput":
            out_names.append(name)
            shape = tuple(alloc.tensor_shape)
            dtype = mybir.dt.np(alloc.dtype)
            out_avals.append(jax.core.ShapedArray(shape, dtype))
            zero_outs.append(np.zeros(shape, dtype))
    n_params = len(in_names)
    all_in_names = list(in_names) + list(out_names)
    if partition_name is not None:
        all_in_names.append(partition_name)
    donate = tuple(range(n_params, n_params + len(out_names)))

    def _body(*args):
        operands = list(args)
        if partition_name is not None:
            operands.append(partition_id_tensor())
        outs = _bass_exec_p.bind(
            *operands, out_avals=tuple(out_avals), in_names=tuple(all_in_names),
            out_names=tuple(out_names), lowering_input_output_aliases=(),
            sim_require_finite=True, sim_require_nnan=True, nc=nc)
        return tuple(outs)

    fn = jax.jit(_body, donate_argnums=donate, keep_unused=True)
    futures = []
    for c, in_map in zip(core_ids, in_maps):
        dev = jax.devices()[c]
        args = [jax.device_put(np.asarray(in_map[nm]), dev) for nm in in_names]
        zz = [jax.device_put(z, dev) for z in zero_outs]
        futures.append(fn(*args, *zz))
    return [{nm: np.asarray(a) for nm, a in zip(out_names, f)} for f in futures]


_PROGRAM_CACHE = {}

MM_DT = "bf16"


def kernel(**inputs):
    os.environ.setdefault("NEURON_COMPILE_CACHE_URL", "/tmp/neuron_cache_kernel")
    x = np.asarray(inputs["x"], dtype=np.float32)
    mask = np.asarray(inputs["mask"]).astype(bool)
    pos_emb = np.asarray(inputs["pos_emb"], dtype=np.float32)
    g = np.asarray(inputs["g"], dtype=np.float32)
    Wq = np.asarray(inputs["Wq"], dtype=np.float32)
    Wkv = np.asarray(inputs["Wkv"], dtype=np.float32)
    Wo = np.asarray(inputs["Wo"], dtype=np.float32)
    bo = np.asarray(inputs["bo"], dtype=np.float32)
    b, n, _ = x.shape
    assert (b, n) == (2, 2048), (b, n)
    mm_dt = MM_DT
    use_km = not bool(mask.all())
    key = (n, mm_dt, use_km)
    if key not in _PROGRAM_CACHE:
        _PROGRAM_CACHE[key] = build_program(n=n, mm_dt=mm_dt, use_kmask=use_km)
    nc = _PROGRAM_CACHE[key]
    core_ids = list(range(8))
    in_maps = [make_core_inputs(x, mask, pos_emb, g, Wq, Wkv, Wo, c, n, mm_dt)
               for c in core_ids]
    results = _run_per_device(nc, in_maps, core_ids)
    out = np.zeros((b, n, D), np.float32)
    for c in core_ids:
        out[c // 4] += results[c]["out"].astype(np.float32)
    out += bo[None, None, :]
    return out


# revision 62
# speedup vs baseline: 1.0067x; 1.0057x over previous
"""Sharded causal attention kernel for trn2 (per-core program builder), v8.

Sharding: 8 cores = 2 batches x 4 head-groups (4 heads each); each core
computes its heads' full attention; host sums the two head-group partial
out-projections per batch.

Per-core structure (bf16 matmuls, fp32 psum):
  - rmsnorm overlapped with the x DMA stream: squares split DVE/Act,
    Act abs_rsqrt replaces sqrt+max+reciprocal, ssq chunks packed two
    per PSUM bank (partition offsets 0/64)
  - rotary fused into the projections: packed [D,128] rotate-half
    weights (4 heads x 32 rot dims), per-token rms scale folded into
    the cos/sin multipliers, combined into qT/kT via gpsimd SWDGE
    DMA accumulate-adds (keeps DVE off the critical path)
  - attention: block-causal at 128-token granularity (diagonal tiles
    trimmed), causal mask applied post-exp as a 0/1 multiply on Es,
    softmax denominator produced by a 64-wide ones block appended to v
    (lands replicated on psum rows 64:128 - no broadcast matmul),
    g-loop software-pipelined so Act runs exps back-to-back
  - k-projection chunks and deferred out-projections are interleaved
    into the attention steps as PE filler; out tiles stored as bf16
  - PSUM plan: norm pools (left) free early for psim; projections+po
    share a right-side pool; pvh+drain-po share the remaining banks
"""

from contextlib import ExitStack

import numpy as np

import concourse.bass as bass
import concourse.mybir as mybir
import concourse.tile as tile
from concourse import bacc

f32 = mybir.dt.float32
f32r = mybir.dt.float32r
bf16 = mybir.dt.bfloat16
AF = mybir.ActivationFunctionType
OP = mybir.AluOpType

D = 1024
HPC = 4
DH = 64
ROT = 32
P = 128
NEG = -1e30


def build_program(n=2048, mm_dt="bf16", use_kmask=False):
    KT = D // P
    NQB = n // 512
    NTOK = n // P
    NCH = n // 512
    mdt = {"f32": f32, "f32r": f32r, "bf16": bf16}[mm_dt]
    nc = bacc.Bacc("TRN2", target_bir_lowering=False, debug=False)

    def din(name, shape, dt_):
        return nc.dram_tensor(name, shape, dt_, kind="ExternalInput")

    xT_d = din("xT", [D, n], mdt)
    # weights come in t-major packed layout [128, KT*cols] (one DMA each)
    wq_d = din("wq", [P, KT * HPC * DH], mdt)
    wk_d = din("wk", [P, KT * HPC * DH], mdt)
    wv_d = din("wv", [P, KT * HPC * DH], mdt)
    wqr_d = din("wqr", [P, KT * P], mdt)   # 4 heads x 32 rot cols per t
    wkr_d = din("wkr", [P, KT * P], mdt)
    wo_d = din("wo", [P, 2 * D], mdt)
    cos_d = din("cos128", [P, n], f32)   # rot rows cos, pass rows 1.0
    sin_d = din("sinc128", [P, n], f32)  # all four 32-row blocks = sin
    tri_d = din("tri01", [P, P], mdt)    # 1.0 where key<=query else 0.0
    id_d = din("ident", [P, P], f32)
    km_d = din("kmask", [P, NTOK], f32) if use_kmask else None
    out_d = nc.dram_tensor("out", [n, D], mdt, kind="ExternalOutput")

    with tile.TileContext(nc) as tc, ExitStack() as top:
        persist = top.enter_context(tc.tile_pool(name="persist", bufs=1))
        ones_f32 = persist.tile([P, 1], f32, name="ones_f32")
        nc.vector.memset(ones_f32, 1.0)
        ones_col = persist.tile([P, 1], mdt, name="ones_col")
        nc.vector.tensor_copy(ones_col, ones_f32)
        ones_row_f = persist.tile([1, P], f32, name="ones_row_f")
        nc.vector.memset(ones_row_f, 1.0)
        ones_row = persist.tile([1, P], f32r, name="ones_row")
        nc.vector.tensor_copy(ones_row, ones_row_f)
        # preload the act table containing Square/AbsRsqrt/Copy so the norm
        # path doesn't eat a mid-phase table switch (Exp set loads later once)
        dummy_act = persist.tile([1, 1], f32, name="dummy_act")
        nc.scalar.activation(dummy_act, ones_f32[0:1, 0:1],
                             AF.Abs_reciprocal_sqrt)

        qkv = top.enter_context(tc.tile_pool(name="qkv", bufs=1))
        qT = [qkv.tile([P, n], mdt, name=f"qT{m}", tag=f"qT{m}") for m in range(2)]
        kT = [qkv.tile([P, n], mdt, name=f"kT{m}", tag=f"kT{m}") for m in range(2)]
        # per head: [64 v-dims | 64 ones]; the ones block makes the pv matmul
        # emit the softmax denominator replicated on psum rows 64:128
        v_sb = [qkv.tile([P, HPC * 2 * DH], mdt, name=f"v{tk}", tag=f"v{tk}")
                for tk in range(NTOK)]
        for tk in range(NTOK):
            vv = v_sb[tk].rearrange("p (h c) -> p h c", h=HPC)
            for hh in range(HPC):
                nc.gpsimd.memset(vv[:, hh, DH:2 * DH], 1.0)
        normk = top.enter_context(tc.tile_pool(name="normk", bufs=1))
        rs_col = normk.tile([P, NTOK], f32, name="rs_col")
        late = top.enter_context(tc.tile_pool(name="late", bufs=3))
        wop = top.enter_context(tc.tile_pool(name="wop", bufs=1))

        big = top.enter_context(tc.tile_pool(name="big", bufs=1))
        # DMA issue order = consumption order (single serialized DMA).
        # x0 lands in 512-col chunks so the first square runs ~1.4us earlier.
        x_sb = [big.tile([P, n], mdt, name=f"x{t}", tag=f"x{t}") for t in range(KT)]
        def wload(dsrc, w_, nm):
            tl = big.tile([P, KT * w_], mdt, name=nm, tag=nm)
            nc.sync.dma_start(out=tl, in_=dsrc[:])
            return [tl[:, t * w_:(t + 1) * w_] for t in range(KT)]

        for c in range(NCH):
            nc.sync.dma_start(out=x_sb[0][:, c * 512:(c + 1) * 512],
                              in_=xT_d[0:P, c * 512:(c + 1) * 512])
        for t in range(1, KT):
            nc.sync.dma_start(out=x_sb[t], in_=xT_d[t * P:(t + 1) * P, :])
        wq = wload(wq_d, HPC * DH, "wq")
        cos_sb = big.tile([P, n], f32, name="cos_sb")
        sin_sb = big.tile([P, n], f32, name="sin_sb")
        nc.sync.dma_start(out=cos_sb, in_=cos_d[:])
        nc.sync.dma_start(out=sin_sb, in_=sin_d[:])
        ident_sb = persist.tile([P, P], f32, name="ident_sb")
        nc.sync.dma_start(out=ident_sb, in_=id_d[:])
        wqr = wload(wqr_d, P, "wqr")
        wv = wload(wv_d, HPC * DH, "wv")
        wk = wload(wk_d, HPC * DH, "wk")
        wkr = wload(wkr_d, P, "wkr")
        wo_all = wop.tile([P, 2 * D], mdt, name="wo_all")
        nc.sync.dma_start(out=wo_all, in_=wo_d[:])
        wo_sb = [wo_all[:, m * D:(m + 1) * D] for m in range(2)]
        tri_sb = persist.tile([P, P], mdt, name="tri_sb")
        nc.sync.dma_start(out=tri_sb, in_=tri_d[:])
        if use_kmask:
            km_sb = persist.tile([P, NTOK], f32, name="km_sb")
            nc.sync.dma_start(out=km_sb, in_=km_d[:])

        usp = top.enter_context(tc.tile_pool(name="usp", bufs=3))

        def rot_combine(base, nm_, c, pss, psr):
            sl = slice(c * 512, (c + 1) * 512)
            for m in range(2):
                nc.vector.tensor_mul(base[m][:, sl], pss[m], cos_sb[:, sl])
            nc.vector.tensor_mul(psr, psr, sin_sb[:, sl])
            u = usp.tile([P, 512], mdt, name=f"u_{nm_}{c}", tag="u")
            # k chunks run during attention where Act is exp-bound: keep
            # the Act queue clear there and stage via DVE instead
            if nm_ == "k":
                nc.vector.tensor_copy(u, psr)
            else:
                nc.scalar.copy(u, psr)
            for h in range(HPC):
                m, h2 = h // 2, h % 2
                nc.gpsimd.dma_start(
                    out=base[m][64 * h2:64 * h2 + ROT, sl],
                    in_=u[ROT * h:ROT * (h + 1), :],
                    accum_op=OP.add)

        # ---- rmsnorm + q/v projections: pnorm/pbc on the left free early
        # so psim/pvp land there; pp (bufs=3) closes before attention ----
        with tc.tile_pool(name="pnorm", bufs=1, space="PSUM") as pnorm, \
             tc.tile_pool(name="pbc", bufs=1, space="PSUM") as pbc, \
             tc.tile_pool(name="pproj", bufs=4, side="right",
                          space="PSUM") as pp, \
             tc.tile_pool(name="normt", bufs=1) as normt, \
             tc.tile_pool(name="sqp", bufs=3) as sqp:
            # matmul PSUM writes must start at partition 0/32/64: two
            # chunks per bank at partitions {0, 64}
            ssq2 = [pnorm.tile([P, 512], f32, name=f"ssq2_{i}", tag=f"ssq2_{i}")
                    for i in range(2)]
            sloc = [(ssq2[c // 2], 64 * (c % 2)) for c in range(NCH)]
            for t in range(KT):
                for c in range(NCH):
                    sq = sqp.tile([P, 512], mdt, name=f"sq{t}_{c}", tag="sq")
                    xs = x_sb[t][:, c * 512:(c + 1) * 512]
                    if (t * NCH + c) % 2 == 0:
                        nc.vector.tensor_mul(sq, xs, xs)
                    else:
                        nc.scalar.activation(sq, xs, AF.Square)
                    stile, soff = sloc[c]
                    nc.tensor.matmul(stile[soff:soff + 1, :], ones_col, sq,
                                     start=(t == 0), stop=(t == KT - 1))
            s_row = normt.tile([1, n], f32r, name="s_row")
            for c in range(NCH):
                sl = slice(c * 512, (c + 1) * 512)
                stile, soff = sloc[c]
                # s = 1/sqrt(ssq/D); matches 1/max(sqrt(.), eps) for all
                # realistic (nonzero) token rows
                with nc.allow_low_precision(reason="f32r has f32 bits"):
                    nc.scalar.activation(s_row[:, sl], stile[soff:soff + 1, :],
                                         AF.Abs_reciprocal_sqrt, scale=1.0 / D)
                bc = pbc.tile([P, 512], f32, name=f"bc{c}", tag="bc")
                nc.tensor.matmul(bc, ones_row, s_row[:, sl],
                                 start=True, stop=True)
                nc.vector.tensor_mul(cos_sb[:, sl], cos_sb[:, sl], bc)
                nc.vector.tensor_mul(sin_sb[:, sl], sin_sb[:, sl], bc)
                for tb in range(4):
                    tk = c * 4 + tb
                    dg = sqp.tile([P, P], f32, name=f"dg_{tk}", tag="dg")
                    nc.vector.tensor_mul(dg, bc[:, tb * P:(tb + 1) * P], ident_sb)
                    nc.vector.reduce_sum(rs_col[:, tk:tk + 1], dg,
                                         axis=mybir.AxisListType.X)

            # ---- q projection ----
            for c in range(NCH):
                sl = slice(c * 512, (c + 1) * 512)
                pss = []
                for m in range(2):
                    ps = pp.tile([P, 512], f32, name=f"ppq{m}_{c}", tag="pp")
                    for t in range(KT):
                        nc.tensor.matmul(ps, wq[t][:, m * P:(m + 1) * P],
                                         x_sb[t][:, sl],
                                         start=(t == 0), stop=(t == KT - 1))
                    pss.append(ps)
                psr = pp.tile([P, 512], f32, name=f"ppqr_{c}", tag="pp")
                for t in range(KT):
                    nc.tensor.matmul(psr, wqr[t], x_sb[t][:, sl],
                                     start=(t == 0), stop=(t == KT - 1))
                rot_combine(qT, "q", c, pss, psr)
            # ---- v projection ----
            for tk in range(NTOK):
                ps = pp.tile([P, HPC * DH], f32, name=f"ppv_{tk}", tag="pp")
                for t in range(KT):
                    nc.tensor.matmul(ps, x_sb[t][:, tk * P:(tk + 1) * P], wv[t],
                                     start=(t == 0), stop=(t == KT - 1))
                vv = v_sb[tk].rearrange("p (h c) -> p h c", h=HPC)
                nc.vector.tensor_scalar_mul(
                    vv[:, :, 0:DH], ps.rearrange("p (h c) -> p h c", h=HPC),
                    rs_col[:, tk:tk + 1])

        # ---- attention + k-projection, finely interleaved ----
        with tc.tile_pool(name="ep", bufs=3) as ep, \
             tc.tile_pool(name="rcpp", bufs=4) as rcpp, \
             tc.tile_pool(name="outsb", bufs=4) as osb, \
             tc.tile_pool(name="psim", bufs=1, space="PSUM") as psim, \
             tc.tile_pool(name="pvp", bufs=2, space="PSUM") as pvp, \
             tc.tile_pool(name="ppk", bufs=2, space="PSUM") as ppk:
            # all AbsRsqrt uses are behind us: preload the Exp table now so
            # the first attention exp doesn't stall on a mid-pipeline load
            nc.scalar.activation(dummy_act, dummy_act, AF.Exp)

            def k_pieces(c):
                sl = slice(c * 512, (c + 1) * 512)
                state = {}

                def piece_m(m):
                    def go():
                        ps = ppk.tile([P, 512], f32, name=f"ppk{m}_{c}",
                                      tag="ppk")
                        for t in range(KT):
                            nc.tensor.matmul(ps, wk[t][:, m * P:(m + 1) * P],
                                             x_sb[t][:, sl],
                                             start=(t == 0), stop=(t == KT - 1))
                        state[m] = ps
                    return go

                def piece_rot():
                    psr = ppk.tile([P, 512], f32, name=f"ppkr_{c}", tag="ppk")
                    for t in range(KT):
                        nc.tensor.matmul(psr, wkr[t], x_sb[t][:, sl],
                                         start=(t == 0), stop=(t == KT - 1))
                    rot_combine(kT, "k", c, [state[0], state[1]], psr)

                return [piece_m(0), piece_m(1), piece_rot]

            pvhs = {}
            attns = {}

            def po_pieces(qb):
                attn = attns[qb]

                def piece_tk(tk):
                    def go():
                        tkl = tk - 4 * qb
                        ob = osb.tile([P, D], mdt, name=f"ob_{tk}", tag="ob")
                        for c2 in range(D // 512):
                            # at the drain (last qb) the pvh slots are free:
                            # alternate pools for a 4-deep po rotation
                            pool = pvp if (qb == NQB - 1 and
                                           (2 * tkl + c2) % 2 == 1) else ppk
                            tg = "pv" if pool is pvp else "ppk"
                            po = pool.tile([P, 512], f32, name=f"po_{tk}_{c2}",
                                           tag=tg)
                            for m in range(2):
                                nc.tensor.matmul(
                                    po, attn[m][:, tkl * P:(tkl + 1) * P],
                                    wo_sb[m][:, c2 * 512:(c2 + 1) * 512],
                                    start=(m == 0), stop=(m == 1))
                            obc = ob[:, c2 * 512:(c2 + 1) * 512]
                            # Act only helps at the drain (qb3), where the
                            # exp stream has ended; elsewhere it would delay
                            # exps which pace the attention stretches
                            if qb == NQB - 1 and c2 == 1:
                                nc.scalar.copy(obc, po)
                            else:
                                nc.vector.tensor_copy(obc, po)
                        nc.sync.dma_start(out=out_d[tk * P:(tk + 1) * P, :],
                                          in_=ob)
                    return go

                return [piece_tk(tk) for tk in range(4 * qb, 4 * qb + 4)]

            def emit_sims(qb, pr, g):
                nkt = 4 * qb + 4
                segs, off = [], 0
                for kt_ in (2 * g, 2 * g + 1):
                    if kt_ >= nkt:
                        continue
                    qlo = max(0, kt_ - 4 * qb) * P
                    segs.append((kt_, qlo, off, 512 - qlo))
                    off += 512 - qlo
                sims = [psim.tile([P, off], f32, name=f"s{h2}_{pr}_{qb}_{g}",
                                  tag=f"sim{h2}") for h2 in range(2)]
                for kt_, qlo, o, w in segs:
                    for h2 in range(2):
                        nc.tensor.matmul(
                            sims[h2][:, o:o + w],
                            kT[pr][64 * h2:64 * h2 + 64, kt_ * P:(kt_ + 1) * P],
                            qT[pr][64 * h2:64 * h2 + 64,
                                   qb * 512 + qlo:(qb + 1) * 512],
                            start=True, stop=True, tile_position=(64 * h2, 0))
                if use_kmask:
                    for kt_, qlo, o, w in segs:
                        for h2 in range(2):
                            sl = sims[h2][:, o:o + w]
                            nc.vector.tensor_scalar_add(sl, sl,
                                                        km_sb[:, kt_:kt_ + 1])
                return sims, segs

            def emit_tail(qb, pr):
                pvh = pvhs[(qb, pr)]
                attns.setdefault(qb, [None, None])
                attns[qb][pr] = late.tile([P, 512], mdt, name=f"attn{pr}_{qb}",
                                          tag=f"attn{pr}")
                at = attns[qb][pr]
                for h2 in range(2):
                    rcp = rcpp.tile([DH, 512], f32, name=f"rcp_{pr}_{qb}_{h2}",
                                    tag="rcp")
                    nc.vector.reciprocal(rcp, pvh[h2][DH:2 * DH, :])
                    nc.vector.tensor_tensor(
                        at[64 * h2:64 * h2 + 64, :], pvh[h2][0:DH, :],
                        rcp, OP.mult)

            def run_group(qb, fillers):
                nkt = 4 * qb + 4
                ng = (nkt + 1) // 2
                steps = [(pr, g) for pr in range(2) for g in range(ng)]
                cur = emit_sims(qb, 0, 0)
                for idx, (pr, g) in enumerate(steps):
                    sims, segs = cur
                    w_ = segs[-1][2] + segs[-1][3]
                    Es = [ep.tile([P, w_], mdt, name=f"E{h2}_{pr}_{qb}_{g}",
                                  tag=f"E{h2}") for h2 in range(2)]
                    for h2 in range(2):
                        nc.scalar.activation(Es[h2], sims[h2], AF.Exp)
                    for kt_, qlo, o, w in segs:
                        if kt_ - 4 * qb >= 0:
                            for h2 in range(2):
                                sl = Es[h2][:, o:o + P]
                                nc.vector.tensor_mul(sl, sl, tri_sb)
                    if idx + 1 < len(steps):
                        cur = emit_sims(qb, *steps[idx + 1])
                    if fillers:
                        fillers.pop(0)()
                    if g == 0:
                        pvhs[(qb, pr)] = [
                            pvp.tile([P, 512], f32, name=f"pv_{pr}_{qb}_{h2}",
                                     tag="pv") for h2 in range(2)]
                    pvh = pvhs[(qb, pr)]
                    for kt_, qlo, o, w in segs:
                        for h2 in range(2):
                            hh = 2 * pr + h2
                            nc.tensor.matmul(
                                pvh[h2][:, qlo:512],
                                v_sb[kt_][:, 2 * DH * hh:2 * DH * hh + 2 * DH],
                                Es[h2][:, o:o + w],
                                start=(kt_ == 0), stop=(kt_ == nkt - 1),
                                skip_group_check=True)
                    if g == ng - 1:
                        emit_tail(qb, pr)
                for f in fillers:
                    f()
                fillers.clear()

            for p_ in k_pieces(0):
                p_()
            run_group(0, k_pieces(1))
            run_group(1, k_pieces(2) + po_pieces(0))
            run_group(2, k_pieces(3) + po_pieces(1))
            run_group(3, po_pieces(2))
            for p_ in po_pieces(3):
                p_()

    nc.compile()
    return nc


# ---------------------------------------------------------------- host side

def np_dt(mm_dt):
    import ml_dtypes
    return {"f32": np.float32, "f32r": np.float32, "bf16": ml_dtypes.bfloat16}[mm_dt]


def _tmajor(W):
    """[D, cols] -> [128, KT*cols] t-major packing for single-DMA load."""
    KT = W.shape[0] // P
    return np.concatenate([W[t * P:(t + 1) * P, :] for t in range(KT)], axis=1)


def make_core_inputs(x, mask, pos_emb, g, Wq, Wkv, Wo, core, n, mm_dt="bf16"):
    ndt = np_dt(mm_dt)
    b = core // 4
    h0 = (core % 4) * HPC
    scale = DH ** -0.5
    gW = Wq * g[:, None]
    gKV = Wkv * g[:, None]
    cols = slice(h0 * DH, (h0 + HPC) * DH)
    wq = gW[:, cols] * scale
    Wk_full = gKV[:, :D]
    Wv_full = gKV[:, D:]
    wk = Wk_full[:, cols]
    wv = Wv_full[:, cols]

    def rot_cols(W):
        # [h0:32 | h1:32 | h2:32 | h3:32] rotate-half columns
        out = np.zeros((D, P), dtype=W.dtype)
        for h in range(HPC):
            src = W[:, (h0 + h) * DH:(h0 + h) * DH + DH]
            base = h * ROT
            out[:, base:base + 16] = -src[:, 16:32]
            out[:, base + 16:base + 32] = src[:, 0:16]
        return out

    wqr = rot_cols(gW) * scale
    wkr = rot_cols(Wk_full)
    wo = np.concatenate([Wo[cols, :][m * P:(m + 1) * P, :] for m in range(2)],
                        axis=1)

    cosf = np.cos(pos_emb.T).astype(np.float32)
    sinf = np.sin(pos_emb.T).astype(np.float32)
    cos128 = np.ones((P, n), np.float32)
    cos128[0:ROT] = cosf
    cos128[DH:DH + ROT] = cosf
    sinc128 = np.empty((P, n), np.float32)
    for h in range(HPC):
        sinc128[h * ROT:(h + 1) * ROT] = sinf
    tri01 = (np.arange(P)[:, None] <= np.arange(P)[None, :]).astype(np.float32)

    ins = {
        "xT": np.ascontiguousarray(x[b].T).astype(ndt),
        "wq": _tmajor(wq).astype(ndt), "wk": _tmajor(wk).astype(ndt),
        "wv": _tmajor(wv).astype(ndt), "wqr": _tmajor(wqr).astype(ndt),
        "wkr": _tmajor(wkr).astype(ndt), "wo": wo.astype(ndt),
        "cos128": cos128, "sinc128": sinc128, "tri01": tri01.astype(ndt),
        "ident": np.eye(P, dtype=np.float32),
    }
    if not mask.all():
        km = np.where(mask[b], 0.0, NEG).astype(np.float32)
        ins["kmask"] = np.ascontiguousarray(km.reshape(n // P, P).T)
    return ins


# ---------------------------------------------------------------- runner

import os
import jax


def _run_per_device(nc, in_maps, core_ids):
    """Run the same Bass program independently on each visible device."""
    from concourse.bass2jax import (_bass_exec_p, install_neuronx_cc_hook,
                                    partition_id_tensor)
    install_neuronx_cc_hook()
    partition_name = nc.partition_id_tensor.name if nc.partition_id_tensor else None
    in_names, out_names, out_avals, zero_outs = [], [], [], []
    for alloc in nc.m.functions[0].allocations:
        if not isinstance(alloc, mybir.MemoryLocationSet):
            continue
        name = alloc.memorylocations[0].name
        if alloc.kind == "ExternalInput":
            if name != partition_name:
                in_names.append(name)
        elif alloc.kind == "ExternalOutput":
            out_names.append(name)
            shape = tuple(alloc.tensor_shape)
            dtype = mybir.dt.np(alloc.dtype)
            out_avals.append(jax.core.ShapedArray(shape, dtype))
            zero_outs.append(np.zeros(shape, dtype))
    n_params = len(in_names)
    all_in_names = list(in_names) + list(out_names)
    if partition_name is not None:
        all_in_names.append(partition_name)
    donate = tuple(range(n_params, n_params + len(out_names)))

    def _body(*args):
        operands = list(args)
        if partition_name is not None:
            operands.append(partition_id_tensor())
        outs = _bass_exec_p.bind(
            *operands, out_avals=tuple(out_avals), in_names=tuple(all_in_names),
            out_names=tuple(out_names), lowering_input_output_aliases=(),
            sim_require_finite=True, sim_require_nnan=True, nc=nc)
        return tuple(outs)

    fn = jax.jit(_body, donate_argnums=donate, keep_unused=True)
    futures = []
    for c, in_map in zip(core_ids, in_maps):
        dev = jax.devices()[c]
        args = [jax.device_put(np.asarray(in_map[nm]), dev) for nm in in_names]
        zz = [jax.device_put(z, dev) for z in zero_outs]
        futures.append(fn(*args, *zz))
    return [{nm: np.asarray(a) for nm, a in zip(out_names, f)} for f in futures]


_PROGRAM_CACHE = {}

MM_DT = "bf16"


def kernel(**inputs):
    os.environ.setdefault("NEURON_COMPILE_CACHE_URL", "/tmp/neuron_cache_kernel")
    x = np.asarray(inputs["x"], dtype=np.float32)
    mask = np.asarray(inputs["mask"]).astype(bool)
    pos_emb = np.asarray(inputs["pos_emb"], dtype=np.float32)
    g = np.asarray(inputs["g"], dtype=np.float32)
    Wq = np.asarray(inputs["Wq"], dtype=np.float32)
    Wkv = np.asarray(inputs["Wkv"], dtype=np.float32)
    Wo = np.asarray(inputs["Wo"], dtype=np.float32)
    bo = np.asarray(inputs["bo"], dtype=np.float32)
    b, n, _ = x.shape
    assert (b, n) == (2, 2048), (b, n)
    mm_dt = MM_DT
    use_km = not bool(mask.all())
    key = (n, mm_dt, use_km)
    if key not in _PROGRAM_CACHE:
        _PROGRAM_CACHE[key] = build_program(n=n, mm_dt=mm_dt, use_kmask=use_km)
    nc = _PROGRAM_CACHE[key]
    core_ids = list(range(8))
    in_maps = [make_core_inputs(x, mask, pos_emb, g, Wq, Wkv, Wo, c, n, mm_dt)
               for c in core_ids]
    results = _run_per_device(nc, in_maps, core_ids)
    out = np.zeros((b, n, D), np.float32)
    for c in core_ids:
        out[c // 4] += results[c]["out"].astype(np.float32)
    out += bo[None, None, :]
    return out


# revision 63
# speedup vs baseline: 1.0070x; 1.0002x over previous
"""Sharded causal attention kernel for trn2 (per-core program builder), v8.

Sharding: 8 cores = 2 batches x 4 head-groups (4 heads each); each core
computes its heads' full attention; host sums the two head-group partial
out-projections per batch.

Per-core structure (bf16 matmuls, fp32 psum):
  - rmsnorm overlapped with the x DMA stream: squares split DVE/Act,
    Act abs_rsqrt replaces sqrt+max+reciprocal, ssq chunks packed two
    per PSUM bank (partition offsets 0/64)
  - rotary fused into the projections: packed [D,128] rotate-half
    weights (4 heads x 32 rot dims), per-token rms scale folded into
    the cos/sin multipliers, combined into qT/kT via gpsimd SWDGE
    DMA accumulate-adds (keeps DVE off the critical path)
  - attention: block-causal at 128-token granularity (diagonal tiles
    trimmed), causal mask applied post-exp as a 0/1 multiply on Es,
    softmax denominator produced by a 64-wide ones block appended to v
    (lands replicated on psum rows 64:128 - no broadcast matmul),
    g-loop software-pipelined so Act runs exps back-to-back
  - k-projection chunks and deferred out-projections are interleaved
    into the attention steps as PE filler; out tiles stored as bf16
  - PSUM plan: norm pools (left) free early for psim; projections+po
    share a right-side pool; pvh+drain-po share the remaining banks
"""

from contextlib import ExitStack

import numpy as np

import concourse.bass as bass
import concourse.mybir as mybir
import concourse.tile as tile
from concourse import bacc

f32 = mybir.dt.float32
f32r = mybir.dt.float32r
bf16 = mybir.dt.bfloat16
AF = mybir.ActivationFunctionType
OP = mybir.AluOpType

D = 1024
HPC = 4
DH = 64
ROT = 32
P = 128
NEG = -1e30


def build_program(n=2048, mm_dt="bf16", use_kmask=False):
    KT = D // P
    NQB = n // 512
    NTOK = n // P
    NCH = n // 512
    mdt = {"f32": f32, "f32r": f32r, "bf16": bf16}[mm_dt]
    nc = bacc.Bacc("TRN2", target_bir_lowering=False, debug=False)

    def din(name, shape, dt_):
        return nc.dram_tensor(name, shape, dt_, kind="ExternalInput")

    xT_d = din("xT", [D, n], mdt)
    # weights come in t-major packed layout [128, KT*cols] (one DMA each)
    wq_d = din("wq", [P, KT * HPC * DH], mdt)
    wk_d = din("wk", [P, KT * HPC * DH], mdt)
    wv_d = din("wv", [P, KT * HPC * DH], mdt)
    wqr_d = din("wqr", [P, KT * P], mdt)   # 4 heads x 32 rot cols per t
    wkr_d = din("wkr", [P, KT * P], mdt)
    wo_d = din("wo", [P, 2 * D], mdt)
    cos_d = din("cos128", [P, n], f32)   # rot rows cos, pass rows 1.0
    sin_d = din("sinc128", [P, n], f32)  # all four 32-row blocks = sin
    tri_d = din("tri01", [P, P], mdt)    # 1.0 where key<=query else 0.0
    id_d = din("ident", [P, P], f32)
    km_d = din("kmask", [P, NTOK], f32) if use_kmask else None
    out_d = nc.dram_tensor("out", [n, D], mdt, kind="ExternalOutput")

    with tile.TileContext(nc) as tc, ExitStack() as top:
        persist = top.enter_context(tc.tile_pool(name="persist", bufs=1))
        ones_f32 = persist.tile([P, 1], f32, name="ones_f32")
        nc.vector.memset(ones_f32, 1.0)
        ones_col = persist.tile([P, 1], mdt, name="ones_col")
        nc.vector.tensor_copy(ones_col, ones_f32)
        ones_row_f = persist.tile([1, P], f32, name="ones_row_f")
        nc.vector.memset(ones_row_f, 1.0)
        ones_row = persist.tile([1, P], f32r, name="ones_row")
        nc.vector.tensor_copy(ones_row, ones_row_f)
        # preload the act table containing Square/AbsRsqrt/Copy so the norm
        # path doesn't eat a mid-phase table switch (Exp set loads later once)
        dummy_act = persist.tile([1, 1], f32, name="dummy_act")
        nc.scalar.activation(dummy_act, ones_f32[0:1, 0:1],
                             AF.Abs_reciprocal_sqrt)

        qkv = top.enter_context(tc.tile_pool(name="qkv", bufs=1))
        qT = [qkv.tile([P, n], mdt, name=f"qT{m}", tag=f"qT{m}") for m in range(2)]
        kT = [qkv.tile([P, n], mdt, name=f"kT{m}", tag=f"kT{m}") for m in range(2)]
        # per head: [64 v-dims | 64 ones]; the ones block makes the pv matmul
        # emit the softmax denominator replicated on psum rows 64:128
        v_sb = [qkv.tile([P, HPC * 2 * DH], mdt, name=f"v{tk}", tag=f"v{tk}")
                for tk in range(NTOK)]
        for tk in range(NTOK):
            vv = v_sb[tk].rearrange("p (h c) -> p h c", h=HPC)
            for hh in range(HPC):
                nc.gpsimd.memset(vv[:, hh, DH:2 * DH], 1.0)
        normk = top.enter_context(tc.tile_pool(name="normk", bufs=1))
        rs_col = normk.tile([P, NTOK], f32, name="rs_col")
        late = top.enter_context(tc.tile_pool(name="late", bufs=4))
        wop = top.enter_context(tc.tile_pool(name="wop", bufs=1))

        big = top.enter_context(tc.tile_pool(name="big", bufs=1))
        # DMA issue order = consumption order (single serialized DMA).
        # x0 lands in 512-col chunks so the first square runs ~1.4us earlier.
        x_sb = [big.tile([P, n], mdt, name=f"x{t}", tag=f"x{t}") for t in range(KT)]
        def wload(dsrc, w_, nm):
            tl = big.tile([P, KT * w_], mdt, name=nm, tag=nm)
            nc.sync.dma_start(out=tl, in_=dsrc[:])
            return [tl[:, t * w_:(t + 1) * w_] for t in range(KT)]

        for c in range(NCH):
            nc.sync.dma_start(out=x_sb[0][:, c * 512:(c + 1) * 512],
                              in_=xT_d[0:P, c * 512:(c + 1) * 512])
        for t in range(1, KT):
            nc.sync.dma_start(out=x_sb[t], in_=xT_d[t * P:(t + 1) * P, :])
        wq = wload(wq_d, HPC * DH, "wq")
        cos_sb = big.tile([P, n], f32, name="cos_sb")
        sin_sb = big.tile([P, n], f32, name="sin_sb")
        nc.sync.dma_start(out=cos_sb, in_=cos_d[:])
        nc.sync.dma_start(out=sin_sb, in_=sin_d[:])
        ident_sb = persist.tile([P, P], f32, name="ident_sb")
        nc.sync.dma_start(out=ident_sb, in_=id_d[:])
        wqr = wload(wqr_d, P, "wqr")
        wv = wload(wv_d, HPC * DH, "wv")
        wk = wload(wk_d, HPC * DH, "wk")
        wkr = wload(wkr_d, P, "wkr")
        wo_all = wop.tile([P, 2 * D], mdt, name="wo_all")
        nc.sync.dma_start(out=wo_all, in_=wo_d[:])
        wo_sb = [wo_all[:, m * D:(m + 1) * D] for m in range(2)]
        tri_sb = persist.tile([P, P], mdt, name="tri_sb")
        nc.sync.dma_start(out=tri_sb, in_=tri_d[:])
        if use_kmask:
            km_sb = persist.tile([P, NTOK], f32, name="km_sb")
            nc.sync.dma_start(out=km_sb, in_=km_d[:])

        usp = top.enter_context(tc.tile_pool(name="usp", bufs=3))

        def rot_combine(base, nm_, c, pss, psr):
            sl = slice(c * 512, (c + 1) * 512)
            for m in range(2):
                nc.vector.tensor_mul(base[m][:, sl], pss[m], cos_sb[:, sl])
            nc.vector.tensor_mul(psr, psr, sin_sb[:, sl])
            u = usp.tile([P, 512], mdt, name=f"u_{nm_}{c}", tag="u")
            # k chunks run during attention where Act is exp-bound: keep
            # the Act queue clear there and stage via DVE instead
            if nm_ == "k":
                nc.vector.tensor_copy(u, psr)
            else:
                nc.scalar.copy(u, psr)
            for h in range(HPC):
                m, h2 = h // 2, h % 2
                nc.gpsimd.dma_start(
                    out=base[m][64 * h2:64 * h2 + ROT, sl],
                    in_=u[ROT * h:ROT * (h + 1), :],
                    accum_op=OP.add)

        # ---- rmsnorm + q/v projections: pnorm/pbc on the left free early
        # so psim/pvp land there; pp (bufs=3) closes before attention ----
        with tc.tile_pool(name="pnorm", bufs=1, space="PSUM") as pnorm, \
             tc.tile_pool(name="pbc", bufs=1, space="PSUM") as pbc, \
             tc.tile_pool(name="pproj", bufs=4, side="right",
                          space="PSUM") as pp, \
             tc.tile_pool(name="normt", bufs=1) as normt, \
             tc.tile_pool(name="sqp", bufs=3) as sqp:
            # matmul PSUM writes must start at partition 0/32/64: two
            # chunks per bank at partitions {0, 64}
            ssq2 = [pnorm.tile([P, 512], f32, name=f"ssq2_{i}", tag=f"ssq2_{i}")
                    for i in range(2)]
            sloc = [(ssq2[c // 2], 64 * (c % 2)) for c in range(NCH)]
            for t in range(KT):
                for c in range(NCH):
                    sq = sqp.tile([P, 512], mdt, name=f"sq{t}_{c}", tag="sq")
                    xs = x_sb[t][:, c * 512:(c + 1) * 512]
                    if (t * NCH + c) % 2 == 0:
                        nc.vector.tensor_mul(sq, xs, xs)
                    else:
                        nc.scalar.activation(sq, xs, AF.Square)
                    stile, soff = sloc[c]
                    nc.tensor.matmul(stile[soff:soff + 1, :], ones_col, sq,
                                     start=(t == 0), stop=(t == KT - 1))
            s_row = normt.tile([1, n], f32r, name="s_row")
            for c in range(NCH):
                sl = slice(c * 512, (c + 1) * 512)
                stile, soff = sloc[c]
                # s = 1/sqrt(ssq/D); matches 1/max(sqrt(.), eps) for all
                # realistic (nonzero) token rows
                with nc.allow_low_precision(reason="f32r has f32 bits"):
                    nc.scalar.activation(s_row[:, sl], stile[soff:soff + 1, :],
                                         AF.Abs_reciprocal_sqrt, scale=1.0 / D)
                bc = pbc.tile([P, 512], f32, name=f"bc{c}", tag="bc")
                nc.tensor.matmul(bc, ones_row, s_row[:, sl],
                                 start=True, stop=True)
                nc.vector.tensor_mul(cos_sb[:, sl], cos_sb[:, sl], bc)
                nc.vector.tensor_mul(sin_sb[:, sl], sin_sb[:, sl], bc)
                for tb in range(4):
                    tk = c * 4 + tb
                    dg = sqp.tile([P, P], f32, name=f"dg_{tk}", tag="dg")
                    nc.vector.tensor_mul(dg, bc[:, tb * P:(tb + 1) * P], ident_sb)
                    nc.vector.reduce_sum(rs_col[:, tk:tk + 1], dg,
                                         axis=mybir.AxisListType.X)

            # ---- q projection ----
            for c in range(NCH):
                sl = slice(c * 512, (c + 1) * 512)
                pss = []
                for m in range(2):
                    ps = pp.tile([P, 512], f32, name=f"ppq{m}_{c}", tag="pp")
                    for t in range(KT):
                        nc.tensor.matmul(ps, wq[t][:, m * P:(m + 1) * P],
                                         x_sb[t][:, sl],
                                         start=(t == 0), stop=(t == KT - 1))
                    pss.append(ps)
                psr = pp.tile([P, 512], f32, name=f"ppqr_{c}", tag="pp")
                for t in range(KT):
                    nc.tensor.matmul(psr, wqr[t], x_sb[t][:, sl],
                                     start=(t == 0), stop=(t == KT - 1))
                rot_combine(qT, "q", c, pss, psr)
            # ---- v projection ----
            for tk in range(NTOK):
                ps = pp.tile([P, HPC * DH], f32, name=f"ppv_{tk}", tag="pp")
                for t in range(KT):
                    nc.tensor.matmul(ps, x_sb[t][:, tk * P:(tk + 1) * P], wv[t],
                                     start=(t == 0), stop=(t == KT - 1))
                vv = v_sb[tk].rearrange("p (h c) -> p h c", h=HPC)
                nc.vector.tensor_scalar_mul(
                    vv[:, :, 0:DH], ps.rearrange("p (h c) -> p h c", h=HPC),
                    rs_col[:, tk:tk + 1])

        # ---- attention + k-projection, finely interleaved ----
        with tc.tile_pool(name="ep", bufs=4) as ep, \
             tc.tile_pool(name="rcpp", bufs=4) as rcpp, \
             tc.tile_pool(name="outsb", bufs=4) as osb, \
             tc.tile_pool(name="psim", bufs=1, space="PSUM") as psim, \
             tc.tile_pool(name="pvp", bufs=2, space="PSUM") as pvp, \
             tc.tile_pool(name="ppk", bufs=2, space="PSUM") as ppk:
            # all AbsRsqrt uses are behind us: preload the Exp table now so
            # the first attention exp doesn't stall on a mid-pipeline load
            nc.scalar.activation(dummy_act, dummy_act, AF.Exp)

            def k_pieces(c):
                sl = slice(c * 512, (c + 1) * 512)
                state = {}

                def piece_m(m):
                    def go():
                        ps = ppk.tile([P, 512], f32, name=f"ppk{m}_{c}",
                                      tag="ppk")
                        for t in range(KT):
                            nc.tensor.matmul(ps, wk[t][:, m * P:(m + 1) * P],
                                             x_sb[t][:, sl],
                                             start=(t == 0), stop=(t == KT - 1))
                        state[m] = ps
                    return go

                def piece_rot():
                    psr = ppk.tile([P, 512], f32, name=f"ppkr_{c}", tag="ppk")
                    for t in range(KT):
                        nc.tensor.matmul(psr, wkr[t], x_sb[t][:, sl],
                                         start=(t == 0), stop=(t == KT - 1))
                    rot_combine(kT, "k", c, [state[0], state[1]], psr)

                return [piece_m(0), piece_m(1), piece_rot]

            pvhs = {}
            attns = {}

            def po_pieces(qb):
                attn = attns[qb]

                def piece_tk(tk):
                    def go():
                        tkl = tk - 4 * qb
                        ob = osb.tile([P, D], mdt, name=f"ob_{tk}", tag="ob")
                        for c2 in range(D // 512):
                            # at the drain (last qb) the pvh slots are free:
                            # alternate pools for a 4-deep po rotation
                            pool = pvp if (qb == NQB - 1 and
                                           (2 * tkl + c2) % 2 == 1) else ppk
                            tg = "pv" if pool is pvp else "ppk"
                            po = pool.tile([P, 512], f32, name=f"po_{tk}_{c2}",
                                           tag=tg)
                            for m in range(2):
                                nc.tensor.matmul(
                                    po, attn[m][:, tkl * P:(tkl + 1) * P],
                                    wo_sb[m][:, c2 * 512:(c2 + 1) * 512],
                                    start=(m == 0), stop=(m == 1))
                            obc = ob[:, c2 * 512:(c2 + 1) * 512]
                            # Act only helps at the drain (qb3), where the
                            # exp stream has ended; elsewhere it would delay
                            # exps which pace the attention stretches
                            if qb == NQB - 1 and c2 == 1:
                                nc.scalar.copy(obc, po)
                            else:
                                nc.vector.tensor_copy(obc, po)
                        nc.sync.dma_start(out=out_d[tk * P:(tk + 1) * P, :],
                                          in_=ob)
                    return go

                return [piece_tk(tk) for tk in range(4 * qb, 4 * qb + 4)]

            def emit_sims(qb, pr, g):
                nkt = 4 * qb + 4
                segs, off = [], 0
                for kt_ in (2 * g, 2 * g + 1):
                    if kt_ >= nkt:
                        continue
                    qlo = max(0, kt_ - 4 * qb) * P
                    segs.append((kt_, qlo, off, 512 - qlo))
                    off += 512 - qlo
                sims = [psim.tile([P, off], f32, name=f"s{h2}_{pr}_{qb}_{g}",
                                  tag=f"sim{h2}") for h2 in range(2)]
                for kt_, qlo, o, w in segs:
                    for h2 in range(2):
                        nc.tensor.matmul(
                            sims[h2][:, o:o + w],
                            kT[pr][64 * h2:64 * h2 + 64, kt_ * P:(kt_ + 1) * P],
                            qT[pr][64 * h2:64 * h2 + 64,
                                   qb * 512 + qlo:(qb + 1) * 512],
                            start=True, stop=True, tile_position=(64 * h2, 0))
                if use_kmask:
                    for kt_, qlo, o, w in segs:
                        for h2 in range(2):
                            sl = sims[h2][:, o:o + w]
                            nc.vector.tensor_scalar_add(sl, sl,
                                                        km_sb[:, kt_:kt_ + 1])
                return sims, segs

            def emit_tail(qb, pr):
                pvh = pvhs[(qb, pr)]
                attns.setdefault(qb, [None, None])
                attns[qb][pr] = late.tile([P, 512], mdt, name=f"attn{pr}_{qb}",
                                          tag=f"attn{pr}")
                at = attns[qb][pr]
                for h2 in range(2):
                    rcp = rcpp.tile([DH, 512], f32, name=f"rcp_{pr}_{qb}_{h2}",
                                    tag="rcp")
                    nc.vector.reciprocal(rcp, pvh[h2][DH:2 * DH, :])
                    nc.vector.tensor_tensor(
                        at[64 * h2:64 * h2 + 64, :], pvh[h2][0:DH, :],
                        rcp, OP.mult)

            def run_group(qb, fillers):
                nkt = 4 * qb + 4
                ng = (nkt + 1) // 2
                steps = [(pr, g) for pr in range(2) for g in range(ng)]
                cur = emit_sims(qb, 0, 0)
                for idx, (pr, g) in enumerate(steps):
                    sims, segs = cur
                    w_ = segs[-1][2] + segs[-1][3]
                    Es = [ep.tile([P, w_], mdt, name=f"E{h2}_{pr}_{qb}_{g}",
                                  tag=f"E{h2}") for h2 in range(2)]
                    for h2 in range(2):
                        nc.scalar.activation(Es[h2], sims[h2], AF.Exp)
                    for kt_, qlo, o, w in segs:
                        if kt_ - 4 * qb >= 0:
                            for h2 in range(2):
                                sl = Es[h2][:, o:o + P]
                                nc.vector.tensor_mul(sl, sl, tri_sb)
                    if idx + 1 < len(steps):
                        cur = emit_sims(qb, *steps[idx + 1])
                    if fillers:
                        fillers.pop(0)()
                    if g == 0:
                        pvhs[(qb, pr)] = [
                            pvp.tile([P, 512], f32, name=f"pv_{pr}_{qb}_{h2}",
                                     tag="pv") for h2 in range(2)]
                    pvh = pvhs[(qb, pr)]
                    for kt_, qlo, o, w in segs:
                        for h2 in range(2):
                            hh = 2 * pr + h2
                            nc.tensor.matmul(
                                pvh[h2][:, qlo:512],
                                v_sb[kt_][:, 2 * DH * hh:2 * DH * hh + 2 * DH],
                                Es[h2][:, o:o + w],
                                start=(kt_ == 0), stop=(kt_ == nkt - 1),
                                skip_group_check=True)
                    if g == ng - 1:
                        emit_tail(qb, pr)
                for f in fillers:
                    f()
                fillers.clear()

            for p_ in k_pieces(0):
                p_()
            run_group(0, k_pieces(1))
            run_group(1, k_pieces(2) + po_pieces(0))
            run_group(2, k_pieces(3) + po_pieces(1))
            run_group(3, po_pieces(2))
            for p_ in po_pieces(3):
                p_()

    nc.compile()
    return nc


# ---------------------------------------------------------------- host side

def np_dt(mm_dt):
    import ml_dtypes
    return {"f32": np.float32, "f32r": np.float32, "bf16": ml_dtypes.bfloat16}[mm_dt]


def _tmajor(W):
    """[D, cols] -> [128, KT*cols] t-major packing for single-DMA load."""
    KT = W.shape[0] // P
    return np.concatenate([W[t * P:(t + 1) * P, :] for t in range(KT)], axis=1)


def make_core_inputs(x, mask, pos_emb, g, Wq, Wkv, Wo, core, n, mm_dt="bf16"):
    ndt = np_dt(mm_dt)
    b = core // 4
    h0 = (core % 4) * HPC
    scale = DH ** -0.5
    gW = Wq * g[:, None]
    gKV = Wkv * g[:, None]
    cols = slice(h0 * DH, (h0 + HPC) * DH)
    wq = gW[:, cols] * scale
    Wk_full = gKV[:, :D]
    Wv_full = gKV[:, D:]
    wk = Wk_full[:, cols]
    wv = Wv_full[:, cols]

    def rot_cols(W):
        # [h0:32 | h1:32 | h2:32 | h3:32] rotate-half columns
        out = np.zeros((D, P), dtype=W.dtype)
        for h in range(HPC):
            src = W[:, (h0 + h) * DH:(h0 + h) * DH + DH]
            base = h * ROT
            out[:, base:base + 16] = -src[:, 16:32]
            out[:, base + 16:base + 32] = src[:, 0:16]
        return out

    wqr = rot_cols(gW) * scale
    wkr = rot_cols(Wk_full)
    wo = np.concatenate([Wo[cols, :][m * P:(m + 1) * P, :] for m in range(2)],
                        axis=1)

    cosf = np.cos(pos_emb.T).astype(np.float32)
    sinf = np.sin(pos_emb.T).astype(np.float32)
    cos128 = np.ones((P, n), np.float32)
    cos128[0:ROT] = cosf
    cos128[DH:DH + ROT] = cosf
    sinc128 = np.empty((P, n), np.float32)
    for h in range(HPC):
        sinc128[h * ROT:(h + 1) * ROT] = sinf
    tri01 = (np.arange(P)[:, None] <= np.arange(P)[None, :]).astype(np.float32)

    ins = {
        "xT": np.ascontiguousarray(x[b].T).astype(ndt),
        "wq": _tmajor(wq).astype(ndt), "wk": _tmajor(wk).astype(ndt),
        "wv": _tmajor(wv).astype(ndt), "wqr": _tmajor(wqr).astype(ndt),
        "wkr": _tmajor(wkr).astype(ndt), "wo": wo.astype(ndt),
        "cos128": cos128, "sinc128": sinc128, "tri01": tri01.astype(ndt),
        "ident": np.eye(P, dtype=np.float32),
    }
    if not mask.all():
        km = np.where(mask[b], 0.0, NEG).astype(np.float32)
        ins["kmask"] = np.ascontiguousarray(km.reshape(n // P, P).T)
    return ins


# ---------------------------------------------------------------- runner

import os
import jax


def _run_per_device(nc, in_maps, core_ids):
    """Run the same Bass program independently on each visible device."""
    from concourse.bass2jax import (_bass_exec_p, install_neuronx_cc_hook,
                                    partition_id_tensor)
    install_neuronx_cc_hook()
    partition_name = nc.partition_id_tensor.name if nc.partition_id_tensor else None
    in_names, out_names, out_avals, zero_outs = [], [], [], []
    for alloc in nc.m.functions[0].allocations:
        if not isinstance(alloc, mybir.MemoryLocationSet):
            continue
        name = alloc.memorylocations[0].name
        if alloc.kind == "ExternalInput":
            if name != partition_name:
                in_names.append(name)
        elif alloc.kind == "ExternalOutput":
            out_names.append(name)
            shape = tuple(alloc.tensor_shape)
            dtype = mybir.dt.np(alloc.dtype)
            out_avals.append(jax.core.ShapedArray(shape, dtype))
            zero_outs.append(np.zeros(shape, dtype))
    n_params = len(in_names)
    all_in_names = list(in_names) + list(out_names)
    if partition_name is not None:
        all_in_names.append(partition_name)
    donate = tuple(range(n_params, n_params + len(out_names)))

    def _body(*args):
        operands = list(args)
        if partition_name is not None:
            operands.append(partition_id_tensor())
        outs = _bass_exec_p.bind(
            *operands, out_avals=tuple(out_avals), in_names=tuple(all_in_names),
            out_names=tuple(out_names), lowering_input_output_aliases=(),
            sim_require_finite=True, sim_require_nnan=True, nc=nc)
        return tuple(outs)

    fn = jax.jit(_body, donate_argnums=donate, keep_unused=True)
    futures = []
    for c, in_map in zip(core_ids, in_maps):
        dev = jax.devices()[c]
        args = [jax.device_put(np.asarray(in_map[nm]), dev) for nm in in_names]
        zz = [jax.device_put(z, dev) for z in zero_outs]
        futures.append(fn(*args, *zz))
    return [{nm: np.asarray(a) for nm, a in zip(out_names, f)} for f in futures]


_PROGRAM_CACHE = {}

MM_DT = "bf16"


def kernel(**inputs):
    os.environ.setdefault("NEURON_COMPILE_CACHE_URL", "/tmp/neuron_cache_kernel")
    x = np.asarray(inputs["x"], dtype=np.float32)
    mask = np.asarray(inputs["mask"]).astype(bool)
    pos_emb = np.asarray(inputs["pos_emb"], dtype=np.float32)
    g = np.asarray(inputs["g"], dtype=np.float32)
    Wq = np.asarray(inputs["Wq"], dtype=np.float32)
    Wkv = np.asarray(inputs["Wkv"], dtype=np.float32)
    Wo = np.asarray(inputs["Wo"], dtype=np.float32)
    bo = np.asarray(inputs["bo"], dtype=np.float32)
    b, n, _ = x.shape
    assert (b, n) == (2, 2048), (b, n)
    mm_dt = MM_DT
    use_km = not bool(mask.all())
    key = (n, mm_dt, use_km)
    if key not in _PROGRAM_CACHE:
        _PROGRAM_CACHE[key] = build_program(n=n, mm_dt=mm_dt, use_kmask=use_km)
    nc = _PROGRAM_CACHE[key]
    core_ids = list(range(8))
    in_maps = [make_core_inputs(x, mask, pos_emb, g, Wq, Wkv, Wo, c, n, mm_dt)
               for c in core_ids]
    results = _run_per_device(nc, in_maps, core_ids)
    out = np.zeros((b, n, D), np.float32)
    for c in core_ids:
        out[c // 4] += results[c]["out"].astype(np.float32)
    out += bo[None, None, :]
    return out
